# revision 1
# baseline (speedup 1.0000x reference)
"""DCRNCognition Trainium2 kernel.

Self-contained: builds a Bass/Tile SPMD program for 8 NeuronCores, shards the
batch (conversation) axis across cores, runs via run_bass_kernel_spmd, and
gathers the valid positions on the host.

Math restructuring (verified vs reference in fp32 numpy, rel err ~9e-7):
  - fc layer folded into step-1 LSTM gates:   gates1 = x @ (w_ih @ fc_w).T + (w_ih@fc_b + b)
  - step-1 has h=c=0: f-gate and the w_hh matmul are dead -> skipped
  - step-2: h1 appears both in qstar and via w_hh:  gates2 = h1 @ (w_ih[:, :D] + w_hh).T + r1 @ w_ih[:, D:].T + b
  - softmax normalization deferred to r:  r = (X^T A) * (1/sum_u A),  A = exp(e + mask)
  - sigmoid computed as 0.5*tanh(x/2)+0.5 so the whole main body uses the
    exp_and_others ACT table set (tanh+exp); h and c are carried scaled by 2
    (hs=2h, cs=2c) with compensations folded into host-side weights and the
    free input-scale of downstream activations.
  - log-softmax head deferred to a final phase (exp+ln share one table set).

Layouts are feature-major (feature dim on SBUF partitions, positions on the
free axis) so LSTM/attention matmuls need no on-device transposes:
  XT  [d, u]   bank (d on partitions)   - lhsT of e-matmul, rhs of gates1
  XN  [u, d]   bank (u on partitions)   - lhsT of r-matmul
  HT  [d, t]   hidden (scaled by 2)     - rhs everywhere
"""
import os
import sys
sys.path.insert(0, '/opt/trn_rl_repo')

# run_bass_kernel_spmd executes through jax/PJRT on the axon-tunneled
# NeuronCores; a JAX_PLATFORMS=cpu pin would hide them.
if os.environ.get('JAX_PLATFORMS') == 'cpu' and 'jax' not in sys.modules:
    del os.environ['JAX_PLATFORMS']

import numpy as np

T_MAX, BATCH, D, C = 512, 128, 256, 7
NCORE = 8
NCONV = BATCH // NCORE          # conversations per core
MASKV = -30000.0                # additive pre-exp mask for invalid bank rows

_BUILD_CACHE = {}


def _build(with_bias1, with_bias2, slot_lens):
    """Build + compile the SPMD Bass program. Returns the Bacc instance."""
    from contextlib import ExitStack
    import concourse.bacc as bacc
    import concourse.bass as bass  # noqa: F401
    from concourse import mybir, tile

    f32 = mybir.dt.float32
    f32r = mybir.dt.float32r
    AF = mybir.ActivationFunctionType
    ALU = mybir.AluOpType

    nc = bacc.Bacc("TRN2", target_bir_lowering=False, debug=False,
                   num_devices=NCORE)

    def din(name, shape):
        return nc.dram_tensor(name, shape, f32, kind="ExternalInput").ap()

    xs_d = din("xs", [T_MAX, NCONV, D])
    xp_d = din("xp", [T_MAX, NCONV, D])
    xst_d = din("xst", [NCONV, 2, 128, T_MAX])   # host-pretransposed d-major banks
    xpt_d = din("xpt", [NCONV, 2, 128, T_MAX])
    mask_d = din("mask", [128, NCONV * 4])
    wdefs = {}
    for st in ("s", "p"):
        wdefs[st] = dict(
            we=din(f"we_{st}", [D, 768]),      # (w_ih@fc_w).T, i/g/o rows only
            wh=din(f"wh_{st}", [D, 1024]),     # 0.5*(w_ih[:, :D] + w_hh).T
            wr=din(f"wr_{st}", [D, 1024]),     # w_ih[:, D:].T
            b1=din(f"b1_{st}", [1, 768]),
            b2=din(f"b2_{st}", [1, 1024]),
        )
    ones_d = din("ones_in", [128, 128])
    outw_d = din("outw", [4 * D, C])           # adjusted out_w.T (h-cols * 0.5)
    outb_d = din("outb", [C, 1])
    out_d = nc.dram_tensor("out", [NCONV, C, T_MAX], f32,
                           kind="ExternalOutput").ap()
    # DRAM scratch for raw logits between main loop and the log-softmax phase
    lg_d = nc.dram_tensor("lg_scratch", [NCONV, C, T_MAX], f32,
                          kind="Internal").ap()

    with ExitStack() as ctx:
        tc = ctx.enter_context(tile.TileContext(nc))
        const = ctx.enter_context(tc.tile_pool(name="const", bufs=1))
        xpool = ctx.enter_context(tc.tile_pool(name="xpool", bufs=2))
        work = ctx.enter_context(tc.tile_pool(name="work", bufs=2))
        fpool = ctx.enter_context(tc.tile_pool(name="fpool", bufs=1))
        lpool = ctx.enter_context(tc.tile_pool(name="lpool", bufs=1))
        gpsum = ctx.enter_context(tc.tile_pool(name="gpsum", bufs=2, space="PSUM"))
        epsum = ctx.enter_context(tc.tile_pool(name="epsum", bufs=2, space="PSUM"))
        spsum = ctx.enter_context(tc.tile_pool(name="spsum", bufs=1, space="PSUM"))
        rpsum = ctx.enter_context(tc.tile_pool(name="rpsum", bufs=1, space="PSUM"))

        # ---- constants / weights --------------------------------------
        W = {}
        for sti, st in enumerate(("s", "p")):
            d = wdefs[st]
            we_t = const.tile([128, 2, 768], f32r, name=f"we_t{st}")
            nc.sync.dma_start(out=we_t, in_=d["we"].bitcast(f32r).rearrange("(kt p) m -> p kt m", p=128))
            wh_t = const.tile([128, 2, 1024], f32r, name=f"wh_t{st}")
            nc.sync.dma_start(out=wh_t, in_=d["wh"].bitcast(f32r).rearrange("(kt p) m -> p kt m", p=128))
            wr_t = const.tile([128, 2, 1024], f32r, name=f"wr_t{st}")
            nc.sync.dma_start(out=wr_t, in_=d["wr"].bitcast(f32r).rearrange("(kt p) m -> p kt m", p=128))
            b1_t = const.tile([1, 768], f32r, name=f"b1_t{st}") if with_bias1 else None
            if with_bias1:
                nc.sync.dma_start(out=b1_t, in_=d["b1"].bitcast(f32r))
            b2_t = const.tile([1, 1024], f32r, name=f"b2_t{st}") if with_bias2 else None
            if with_bias2:
                nc.sync.dma_start(out=b2_t, in_=d["b2"].bitcast(f32r))
            W[sti] = dict(we=we_t, wh=wh_t, wr=wr_t, b1=b1_t, b2=b2_t)
        ones = const.tile([128, 128], f32r)
        nc.sync.dma_start(out=ones, in_=ones_d.bitcast(f32r))
        if with_bias1 or with_bias2:
            onesrow = const.tile([1, T_MAX], f32r)
            nc.sync.dma_start(
                out=onesrow,
                in_=ones_d.bitcast(f32r).rearrange("a b -> (a b)")[0:T_MAX])

        mask_t = const.tile([128, NCONV * 4], f32)
        nc.sync.dma_start(out=mask_t, in_=mask_d)
        outw_t = const.tile([128, 8, C], f32r)
        nc.sync.dma_start(out=outw_t, in_=outw_d.bitcast(f32r).rearrange("(kt p) c -> p kt c", p=128))
        outb_t = const.tile([C, 1], f32)
        nc.sync.dma_start(out=outb_t, in_=outb_d)

        def mm(ps, lhsT, rhs, start, stop):
            nc.tensor.matmul(ps, lhsT, rhs, start=start, stop=stop)

        def attention(j, st, h_t, step, L, UT, prems):
            """A = exp(0.5*e + mask); returns (A tile, Z tile)."""
            A = work.tile([128, 4, T_MAX], f32r, tag="A", name=f"A{j}_{st}_{step}")
            xt = XT[st]
            for ut in range(UT):
                prem = prems[ut]
                pe = epsum.tile([128, T_MAX], f32, tag="pe", name=f"pe{j}_{st}_{step}_{ut}")
                for kd in range(2):
                    mm(pe[0:prem, 0:L], xt[:, kd, ut * 128:ut * 128 + prem],
                       h_t[:, kd, 0:L], kd == 0, kd == 1)
                col = j * 4 + ut
                nc.scalar.activation(A[0:prem, ut, 0:L], pe[0:prem, 0:L], AF.Exp,
                                     bias=mask_t[0:prem, col:col + 1], scale=0.5)
            psm = spsum.tile([128, T_MAX], f32, tag="psm", name=f"psm{j}_{st}_{step}")
            for ut in range(UT):
                prem = prems[ut]
                mm(psm[:, 0:L], ones[0:prem, :], A[0:prem, ut, 0:L],
                   ut == 0, ut == UT - 1)
            Z = work.tile([128, T_MAX], f32, tag="Z", name=f"Z{j}_{st}_{step}")
            nc.vector.reciprocal(Z[:, 0:L], psm[:, 0:L])
            return A, Z

        def r_matmul(j, st, A, Z, out_tile, out_zoff, relu, L, UT, prems):
            """out[:, out_zoff+dt, :] = (X^T A) * Z  (optionally relu'd)."""
            xn = XN[st]
            for dt in range(2):
                pr = rpsum.tile([128, T_MAX], f32, tag="pr", name=f"pr{j}_{st}_{out_zoff}_{dt}")
                for ut in range(UT):
                    prem = prems[ut]
                    mm(pr[:, 0:L], xn[0:prem, ut, dt * 128:(dt + 1) * 128],
                       A[0:prem, ut, 0:L], ut == 0, ut == UT - 1)
                if relu:
                    tmpr = work.tile([128, T_MAX], f32, tag="tmpr", name=f"tmpr{j}_{st}_{dt}")
                    nc.vector.tensor_mul(tmpr[:, 0:L], pr[:, 0:L], Z[:, 0:L])
                    nc.vector.tensor_scalar_max(out_tile[:, out_zoff + dt, 0:L],
                                                tmpr[:, 0:L], 0.0)
                else:
                    nc.vector.tensor_mul(out_tile[:, out_zoff + dt, 0:L],
                                         pr[:, 0:L], Z[:, 0:L])

        # ---- main loop (streams phase-interleaved to keep PE fed) ----
        for j in range(NCONV):
            Lv = int(slot_lens[j])
            # fp32r matmuls need aligned sizes; pad to 8 (pads read real data)
            L = min(T_MAX, ((Lv + 7) // 8) * 8)
            UT = (Lv + 127) // 128
            prems = [min(512 - ut * 128, 128, ((Lv - ut * 128 + 7) // 8) * 8)
                     for ut in range(UT)]
            XT, XN = {}, {}
            g1_, cs1_, hs1_, A1_, Z1_, r1_ = {}, {}, {}, {}, {}, {}
            g2_, cs2_, hs2_, ft_ = {}, {}, {}, {}
            for st in (0, 1):
                src_ = xs_d if st == 0 else xp_d
                srct = xst_d if st == 0 else xpt_d
                xn = xpool.tile([128, 4, D], f32r, tag="xn", name=f"xn{j}_{st}")
                for ut in range(UT):
                    prem = prems[ut]
                    nc.sync.dma_start(
                        out=xn[0:prem, ut, :],
                        in_=src_.bitcast(f32r)[ut * 128:ut * 128 + prem, j, :])
                xt = xpool.tile([128, 2, T_MAX], f32r, tag="xt", name=f"xt{j}_{st}")
                for kd in range(2):
                    nc.sync.dma_start(out=xt[:, kd, 0:L],
                                      in_=srct.bitcast(f32r)[j, kd, :, 0:L])
                XT[st], XN[st] = xt, xn
            for st in (0, 1):
                w = W[st]
                xt = XT[st]
                g1 = {}
                for pi, nm in enumerate(("i", "g", "o")):
                    ps = gpsum.tile([128, 2, T_MAX], f32, tag="pg", name=f"pg1{j}_{st}_{pi}")
                    for z in range(2):
                        m = pi * 2 + z
                        for kd in range(2):
                            mm(ps[:, z, 0:L], w["we"][:, kd, m * 128:(m + 1) * 128],
                               xt[:, kd, 0:L], kd == 0, (kd == 1) and not with_bias1)
                        if with_bias1:
                            mm(ps[:, z, 0:L], w["b1"][:, m * 128:(m + 1) * 128],
                               onesrow[0:1, 0:L], False, True)
                    tt = work.tile([128, 2, T_MAX], f32, tag="gact", bufs=6,
                                   name=f"t1{nm}{j}_{st}")
                    nc.scalar.activation(tt[:, :, 0:L], ps[:, :, 0:L], AF.Tanh,
                                         scale=1.0 if nm == "g" else 0.5)
                    g1[nm] = tt
                g1_[st] = g1
            for st in (0, 1):
                g1 = g1_[st]
                cs1 = work.tile([128, 2, T_MAX], f32, tag="cs", bufs=3, name=f"cs1{j}_{st}")
                nc.vector.scalar_tensor_tensor(cs1[:, :, 0:L], g1["i"][:, :, 0:L],
                                               1.0, g1["g"][:, :, 0:L],
                                               ALU.add, ALU.mult)
                th1 = work.tile([128, 2, T_MAX], f32, tag="tmp", bufs=5, name=f"th1{j}_{st}")
                nc.scalar.activation(th1[:, :, 0:L], cs1[:, :, 0:L], AF.Tanh, scale=0.5)
                hs1 = work.tile([128, 2, T_MAX], f32r, tag="hs", bufs=4, name=f"hs1{j}_{st}")
                nc.vector.scalar_tensor_tensor(hs1[:, :, 0:L], g1["o"][:, :, 0:L],
                                               1.0, th1[:, :, 0:L],
                                               ALU.add, ALU.mult)
                cs1_[st], hs1_[st] = cs1, hs1
            for st in (0, 1):
                A1, Z1 = attention(j, st, hs1_[st], 1, L, UT, prems)
                A1_[st], Z1_[st] = A1, Z1
            for st in (0, 1):
                r1 = work.tile([128, 2, T_MAX], f32r, tag="r1", name=f"r1{j}_{st}")
                r_matmul(j, st, A1_[st], Z1_[st], r1, 0, False, L, UT, prems)
                r1_[st] = r1
            for st in (0, 1):
                w = W[st]
                g2 = {}
                for pi, nm in enumerate(("i", "f", "g", "o")):
                    ps = gpsum.tile([128, 2, T_MAX], f32, tag="pg", name=f"pg2{j}_{st}_{pi}")
                    for z in range(2):
                        m = pi * 2 + z
                        for kd in range(2):
                            mm(ps[:, z, 0:L], w["wh"][:, kd, m * 128:(m + 1) * 128],
                               hs1_[st][:, kd, 0:L], kd == 0, False)
                        for kd in range(2):
                            mm(ps[:, z, 0:L], w["wr"][:, kd, m * 128:(m + 1) * 128],
                               r1_[st][:, kd, 0:L], False, (kd == 1) and not with_bias2)
                        if with_bias2:
                            mm(ps[:, z, 0:L], w["b2"][:, m * 128:(m + 1) * 128],
                               onesrow[0:1, 0:L], False, True)
                    tt = work.tile([128, 2, T_MAX], f32, tag="gact", bufs=6,
                                   name=f"t2{nm}{j}_{st}")
                    nc.scalar.activation(tt[:, :, 0:L], ps[:, :, 0:L], AF.Tanh,
                                         scale=1.0 if nm == "g" else 0.5)
                    g2[nm] = tt
                g2_[st] = g2
            for st in (0, 1):
                g2, cs1 = g2_[st], cs1_[st]
                t1 = work.tile([128, 2, T_MAX], f32, tag="tmp", bufs=5, name=f"t1_{j}_{st}")
                nc.vector.scalar_tensor_tensor(t1[:, :, 0:L], g2["f"][:, :, 0:L], 1.0,
                                               cs1[:, :, 0:L], ALU.add, ALU.mult)
                t2 = work.tile([128, 2, T_MAX], f32, tag="tmp", bufs=5, name=f"t2_{j}_{st}")
                nc.vector.scalar_tensor_tensor(t2[:, :, 0:L], g2["i"][:, :, 0:L], 1.0,
                                               g2["g"][:, :, 0:L], ALU.add, ALU.mult)
                cs2 = work.tile([128, 2, T_MAX], f32, tag="cs", bufs=3, name=f"cs2{j}_{st}")
                nc.vector.scalar_tensor_tensor(cs2[:, :, 0:L], t1[:, :, 0:L], 0.5,
                                               t2[:, :, 0:L], ALU.mult, ALU.add)
                th2 = work.tile([128, 2, T_MAX], f32, tag="tmp", bufs=5, name=f"th2{j}_{st}")
                nc.scalar.activation(th2[:, :, 0:L], cs2[:, :, 0:L], AF.Tanh, scale=0.5)
                hs2 = work.tile([128, 2, T_MAX], f32r, tag="hs", bufs=4, name=f"hs2{j}_{st}")
                nc.vector.scalar_tensor_tensor(hs2[:, :, 0:L], g2["o"][:, :, 0:L],
                                               1.0, th2[:, :, 0:L], ALU.add, ALU.mult)
                hs2_[st] = hs2
            for st in (0, 1):
                A2, Z2 = attention(j, st, hs2_[st], 2, L, UT, prems)
                ft = fpool.tile([128, 4, T_MAX], f32r, tag=f"feat{st}", name=f"feat{j}_{st}")
                for zz in range(2):
                    nc.vector.tensor_scalar_max(ft[:, zz, 0:L],
                                                hs2_[st][:, zz, 0:L].bitcast(f32), 0.0)
                r_matmul(j, st, A2, Z2, ft, 2, True, L, UT, prems)
                ft_[st] = ft

            # ---- logits for conversation j ------------------------------
            pl = gpsum.tile([C, T_MAX], f32, tag="pg", name=f"pl{j}")
            for kt in range(8):
                rhs = ft_[kt // 4][:, kt % 4, 0:L]
                mm(pl[:, 0:L], outw_t[:, kt, :], rhs, kt == 0, kt == 7)
            lg = lpool.tile([C, T_MAX], f32, tag="lg", bufs=2, name=f"lg{j}")
            nc.scalar.activation(lg[:, 0:L], pl[:, 0:L], AF.Copy)
            nc.sync.dma_start(out=lg_d[j, :, 0:L], in_=lg[:, 0:L])

        # ---- log-softmax phase (exp+ln live in one ACT table set) -------
        tc.strict_bb_all_engine_barrier()
        for j in range(NCONV):
            L = min(T_MAX, ((int(slot_lens[j]) + 7) // 8) * 8)
            lgj = lpool.tile([C, T_MAX], f32, tag="lgj", bufs=1, name=f"lgj{j}")
            nc.sync.dma_start(out=lgj[:, 0:L], in_=lg_d[j, :, 0:L])
            elg = lpool.tile([C, T_MAX], f32r, tag="elg", bufs=1, name=f"elg{j}")
            nc.scalar.activation(elg[:, 0:L], lgj[:, 0:L], AF.Exp, bias=outb_t[:, 0:1])
            pls = spsum.tile([128, T_MAX], f32, tag="psm", name=f"pls{j}")
            mm(pls[:, 0:L], ones[0:C, :], elg[:, 0:L], True, True)
            lns = lpool.tile([C, T_MAX], f32, tag="lns", bufs=1, name=f"lns{j}")
            nc.scalar.activation(lns[:, 0:L], pls[0:C, 0:L], AF.Ln)
            lp = lpool.tile([C, T_MAX], f32, tag="lp", bufs=1, name=f"lp{j}")
            nc.vector.scalar_tensor_tensor(lp[:, 0:L], lgj[:, 0:L], outb_t[:, 0:1],
                                           lns[:, 0:L], ALU.add, ALU.subtract)
            nc.sync.dma_start(out=out_d[j, :, 0:L], in_=lp[:, 0:L])

    nc.compile()
    return nc


def _host_prep(inputs):
    """Fold weights, pick the conversation->core assignment, build per-core arrays."""
    x_s = np.ascontiguousarray(np.asarray(inputs["input"], dtype=np.float32))
    x_p = np.ascontiguousarray(np.asarray(inputs["speakers"], dtype=np.float32))
    lengths = np.asarray(inputs["utterance_lengths"]).astype(np.int64)
    fc_w = np.asarray(inputs["fc_w"], dtype=np.float32)
    fc_b = np.asarray(inputs["fc_b"], dtype=np.float32)
    out_w = np.asarray(inputs["out_w"], dtype=np.float32)
    out_b = np.asarray(inputs["out_b"], dtype=np.float32)

    per_stream = {}
    any_b1 = False
    any_b2 = False
    for st in ("s", "p"):
        w_ih = np.asarray(inputs[f"w_ih_{st}"], dtype=np.float32)
        w_hh = np.asarray(inputs[f"w_hh_{st}"], dtype=np.float32)
        b_ih = np.asarray(inputs[f"b_ih_{st}"], dtype=np.float32)
        b_hh = np.asarray(inputs[f"b_hh_{st}"], dtype=np.float32)
        W_eff = w_ih @ fc_w                          # [1024, 256]
        bias1 = w_ih @ fc_b + b_ih + b_hh            # [1024]
        sel = np.r_[0:D, 2 * D:4 * D]                # i, g, o rows
        We_sel = np.ascontiguousarray(W_eff[sel].T)  # [256, 768]
        b1_sel = np.ascontiguousarray(bias1[sel])[None, :]   # [1, 768]
        Wh = np.ascontiguousarray((0.5 * (w_ih[:, :D] + w_hh)).T)  # [256, 1024]
        Wr = np.ascontiguousarray(w_ih[:, D:].T)     # [256, 1024]
        b2 = np.ascontiguousarray(b_ih + b_hh)[None, :]      # [1, 1024]
        per_stream[st] = (We_sel, Wh, Wr, b1_sel, b2)
        any_b1 |= bool(np.any(b1_sel != 0.0))
        any_b2 |= bool(np.any(b2 != 0.0))

    # out_w columns for the h-halves get the 0.5 compensation (h is stored as 2h)
    ow = out_w.copy()
    ow[:, 0:D] *= 0.5
    ow[:, 2 * D:3 * D] *= 0.5
    outw = np.ascontiguousarray(ow.T)                # [1024, 7]
    outb = np.ascontiguousarray(out_b)[:, None]      # [7, 1]

    # conversation -> (core, slot): sort by length desc, round-robin over cores
    order = np.argsort(-lengths, kind="stable")
    assign = {}   # conv -> (core, slot)
    for rank, conv in enumerate(order):
        assign[int(conv)] = (rank % NCORE, rank // NCORE)

    order_lens = lengths[order]
    slot_lens = tuple(int(order_lens[8 * k]) for k in range(NCONV))

    in_maps = []
    core_convs = []
    for core in range(NCORE):
        ids = [None] * NCONV
        for conv, (c, s) in assign.items():
            if c == core:
                ids[s] = conv
        core_convs.append(ids)
        mask = np.zeros((128, NCONV * 4), dtype=np.float32)
        for s, conv in enumerate(ids):
            L = int(lengths[conv])
            u = np.arange(T_MAX)
            m = np.where(u < L, 0.0, MASKV).astype(np.float32)
            mask[:, s * 4:(s + 1) * 4] = m.reshape(4, 128).T
        im = {
            "xs": np.ascontiguousarray(x_s[:, ids, :]),
            "xp": np.ascontiguousarray(x_p[:, ids, :]),
            "xst": np.ascontiguousarray(
                x_s[:, ids, :].transpose(1, 2, 0).reshape(NCONV, 2, 128, T_MAX)),
            "xpt": np.ascontiguousarray(
                x_p[:, ids, :].transpose(1, 2, 0).reshape(NCONV, 2, 128, T_MAX)),
            "mask": mask,
            "ones_in": np.ones((128, 128), dtype=np.float32),
            "outw": outw,
            "outb": outb,
        }
        for st in ("s", "p"):
            We_sel, Wh, Wr, b1_sel, b2 = per_stream[st]
            im[f"we_{st}"] = We_sel
            im[f"wh_{st}"] = Wh
            im[f"wr_{st}"] = Wr
            im[f"b1_{st}"] = b1_sel
            im[f"b2_{st}"] = b2
        in_maps.append(im)
    return in_maps, core_convs, lengths, any_b1, any_b2, slot_lens


def _gather(results, core_convs, lengths):
    """results: list (per core) of {'out': [NCONV, C, T_MAX]} -> [sum(len), C]."""
    where = {}
    for core, ids in enumerate(core_convs):
        for slot, conv in enumerate(ids):
            where[conv] = (core, slot)
    chunks = []
    for b in range(BATCH):
        core, slot = where[b]
        L = int(lengths[b])
        chunks.append(np.ascontiguousarray(results[core]["out"][slot, :, :L].T))
    return np.concatenate(chunks, axis=0).astype(np.float32)


def _get_nc(any_b1, any_b2, slot_lens):
    key = (any_b1, any_b2, slot_lens)
    if key not in _BUILD_CACHE:
        _BUILD_CACHE[key] = _build(any_b1, any_b2, slot_lens)
    return _BUILD_CACHE[key]


def kernel(**inputs):
    from concourse import bass_utils
    in_maps, core_convs, lengths, any_b1, any_b2, slot_lens = _host_prep(inputs)
    nc = _get_nc(any_b1, any_b2, slot_lens)
    res = bass_utils.run_bass_kernel_spmd(nc, in_maps, core_ids=list(range(NCORE)))
    return _gather(res.results, core_convs, lengths)



# revision 10
# speedup vs baseline: 1.0434x; 1.0434x over previous
"""DCRNCognition Trainium2 kernel — fp8 DoubleRow edition.

Self-contained: builds a Bass/Tile SPMD program for 8 NeuronCores, shards the
batch (conversation) axis across cores, runs via run_bass_kernel_spmd, and
gathers the valid positions on the host.

Math restructuring (identical to the verified baseline, rel err ~9e-7 in f32):
  - fc layer folded into step-1 LSTM gates; step-1 f-gate/c-init dead
  - step-2: gates2 = hs1 @ Wh.T + r1 @ Wr.T  (Wh = 0.5*(w_ih[:, :D]+w_hh))
  - softmax normalization deferred to r:  r = (X^T A) * (1/sum_u A)
  - sigmoid via tanh; h,c carried scaled by 2 (hs=2h, cs=2c)

Precision plan (validated on host: rel err ~6e-4 vs the 2e-2 gate):
  - ALL matmuls in fp8 e4m3 with DoubleRow perf mode (2 k-tiles per
    instruction, 0.5 cycles/row) accumulating in fp32 PSUM.
  - weights pre-scaled by a power of 2 into fp8 range on the host; the
    compensation is folded into the (free) activation scale operands.
  - g-gate weight rows doubled on host so every gate activation shares
    scale=0.5 -> whole gate groups convert with ONE ACT instruction.
  - element-wise intermediates in bf16 (DVE 2x/4x perf modes).
  - 1/sum via reciprocal_approx_fast; softmax apply on GPSIMD/DVE.
  - logits computed transposed ([t,C]) so log-softmax reduces along the
    free axis: exp+row-sum per conv in the main loop (exp shares the
    tanh table set), one batched Ln at the end -> 2 table loads total.
"""
import os
import sys
sys.path.insert(0, '/opt/trn_rl_repo')

# run_bass_kernel_spmd executes through jax/PJRT on the axon-tunneled
# NeuronCores; a JAX_PLATFORMS=cpu pin would hide them.
if os.environ.get('JAX_PLATFORMS') == 'cpu' and 'jax' not in sys.modules:
    del os.environ['JAX_PLATFORMS']

import math
import numpy as np

T_MAX, BATCH, D, C = 512, 128, 256, 7
NCORE = 8
NCONV = BATCH // NCORE          # conversations per core
MASKV = -30000.0                # additive pre-exp mask for invalid bank rows

_BUILD_CACHE = {}


def _f8(x):
    """Host fp32 -> e4m3 bytes (clipped to the TRN-compatible +-240 range)."""
    import ml_dtypes
    return np.ascontiguousarray(
        np.clip(np.asarray(x, np.float32), -240.0, 240.0)
        .astype(ml_dtypes.float8_e4m3fn).view(np.uint8))


def _pow2_scale(w):
    s = float(np.std(w))
    if s == 0.0 or not np.isfinite(s):
        return 1.0
    return float(2.0 ** round(math.log2(4.0 / s)))


def _build(with_bias, slot_lens, scales):
    """Build + compile the SPMD Bass program. Returns the Bacc instance."""
    from contextlib import ExitStack
    import concourse.bacc as bacc
    import concourse.bass as bass  # noqa: F401
    from concourse import mybir, tile

    f32 = mybir.dt.float32
    bf16 = mybir.dt.bfloat16
    fp8 = mybir.dt.float8e4
    u8 = mybir.dt.uint8
    AF = mybir.ActivationFunctionType
    ALU = mybir.AluOpType
    AX = mybir.AxisListType
    PM = mybir.MatmulPerfMode.DoubleRow

    ws_e = {0: scales['ws_e_s'], 1: scales['ws_e_p']}
    ws_h = {0: scales['ws_h_s'], 1: scales['ws_h_p']}
    ows = scales['ows']

    nc = bacc.Bacc("TRN2", target_bir_lowering=False, debug=False,
                   num_devices=NCORE)

    def din(name, shape, dt):
        return nc.dram_tensor(name, shape, dt, kind="ExternalInput").ap()

    xt_d = {0: din("xts", [NCONV, 2, 128, T_MAX], u8),
            1: din("xtp", [NCONV, 2, 128, T_MAX], u8)}
    xn_d = {0: din("xns", [T_MAX, NCONV, D], u8),
            1: din("xnp", [T_MAX, NCONV, D], u8)}
    wdefs = {}
    for sti, st in enumerate(("s", "p")):
        wdefs[sti] = dict(
            we=din(f"we_{st}", [D, 768], u8),     # (w_ih@fc_w).T scaled, i/g/o
            wh=din(f"wh_{st}", [D, 1024], u8),    # 0.5*(w_ih[:, :D]+w_hh).T scaled
            wr=din(f"wr_{st}", [D, 1024], u8),    # w_ih[:, D:].T scaled
            b1=din(f"b1_{st}", [128, 6], f32) if with_bias else None,
            b2=din(f"b2_{st}", [128, 8], f32) if with_bias else None,
        )
    ones_d = din("ones8", [128, 256], u8)
    mask_d = din("mask", [128, NCONV * 4], f32)
    outw_d = din("outw", [4 * D, C], u8)          # adjusted out_w.T scaled
    out_d = nc.dram_tensor("out", [NCONV, T_MAX, C], f32,
                           kind="ExternalOutput").ap()
    dbg = os.environ.get('BASSDBG')
    if dbg:
        dbg_lg = nc.dram_tensor("dbg_lg", [128, NCONV, 4, C], f32,
                                kind="ExternalOutput").ap()
        dbg_s = nc.dram_tensor("dbg_s", [128, NCONV * 4], f32,
                               kind="ExternalOutput").ap()
        dbg_ft = nc.dram_tensor("dbg_ft", [NCONV, 2, 128, 4, T_MAX], u8,
                                kind="ExternalOutput").ap()
        dbg_hs = nc.dram_tensor("dbg_hs", [NCONV, 2, 2, 128, 2, T_MAX], u8,
                                kind="ExternalOutput").ap()
        dbg_A = nc.dram_tensor("dbg_A", [NCONV, 2, 2, 128, 4, T_MAX], u8,
                               kind="ExternalOutput").ap()
        dbg_Z = nc.dram_tensor("dbg_Z", [NCONV, 2, 2, 128, T_MAX], f32,
                               kind="ExternalOutput").ap()

    with ExitStack() as ctx:
        tc = ctx.enter_context(tile.TileContext(nc))
        const = ctx.enter_context(tc.tile_pool(name="const", bufs=1))
        xpool = ctx.enter_context(tc.tile_pool(name="xpool", bufs=4))
        work = ctx.enter_context(tc.tile_pool(name="work", bufs=2))
        fpool = ctx.enter_context(tc.tile_pool(name="fpool", bufs=2))
        opool = ctx.enter_context(tc.tile_pool(name="opool", bufs=2))
        gp = ctx.enter_context(tc.tile_pool(name="gp", bufs=1, space="PSUM"))
        ep = ctx.enter_context(tc.tile_pool(name="ep", bufs=1, space="PSUM"))
        rp = ctx.enter_context(tc.tile_pool(name="rp", bufs=1, space="PSUM"))
        lp = ctx.enter_context(tc.tile_pool(name="lp", bufs=1, space="PSUM"))

        # ---- constants / weights --------------------------------------
        W = {}
        for sti, st in enumerate(("s", "p")):
            d = wdefs[sti]
            we_t = const.tile([128, 2, 768], fp8, name=f"we_t{st}")
            nc.sync.dma_start(out=we_t, in_=d["we"].bitcast(fp8).rearrange(
                "(kt p) m -> p kt m", p=128))
            wh_t = const.tile([128, 2, 1024], fp8, name=f"wh_t{st}")
            nc.sync.dma_start(out=wh_t, in_=d["wh"].bitcast(fp8).rearrange(
                "(kt p) m -> p kt m", p=128))
            wr_t = const.tile([128, 2, 1024], fp8, name=f"wr_t{st}")
            nc.sync.dma_start(out=wr_t, in_=d["wr"].bitcast(fp8).rearrange(
                "(kt p) m -> p kt m", p=128))
            b1_t = b2_t = None
            if with_bias:
                b1_t = const.tile([128, 6], f32, name=f"b1_t{st}")
                nc.sync.dma_start(out=b1_t, in_=d["b1"])
                b2_t = const.tile([128, 8], f32, name=f"b2_t{st}")
                nc.sync.dma_start(out=b2_t, in_=d["b2"])
            W[sti] = dict(we=we_t, wh=wh_t, wr=wr_t, b1=b1_t, b2=b2_t)
        ones8 = const.tile([128, 2, 128], fp8, name="ones8")
        nc.sync.dma_start(out=ones8, in_=ones_d.bitcast(fp8).rearrange(
            "p (a b) -> p a b", a=2))
        mask_t = const.tile([128, NCONV * 4], f32, name="mask_t")
        nc.sync.dma_start(out=mask_t, in_=mask_d)
        outw_t = const.tile([128, 8, C], fp8, name="outw_t")
        nc.sync.dma_start(out=outw_t, in_=outw_d.bitcast(fp8).rearrange(
            "(kt p) c -> p kt c", p=128))
        lg_all = const.tile([128, NCONV, 4, C], f32, name="lg_all")
        s_all = const.tile([128, NCONV * 4], f32, name="s_all")
        lnS = const.tile([128, NCONV * 4], f32, name="lnS")

        def mm(ps, lhsT, rhs, start, stop, pm=PM):
            nc.tensor.matmul(ps, lhsT, rhs, start=start, stop=stop,
                             perf_mode=pm)

        # ---- main loop -------------------------------------------------
        for j in range(NCONV):
            Lv = int(slot_lens[j])
            L = min(T_MAX, ((Lv + 7) // 8) * 8)
            UT = (Lv + 127) // 128
            NPAIR, ODD = UT // 2, UT % 2

            XT, XN = {}, {}
            LX = UT * 128   # e-matmul lhsT reads full 128-col blocks; load them
            for st in (0, 1):
                xt = xpool.tile([128, 2, T_MAX], fp8, tag="xt", name=f"xt{j}_{st}")
                for kd in range(2):
                    nc.sync.dma_start(out=xt[:, kd, 0:LX],
                                      in_=xt_d[st].bitcast(fp8)[j, kd, :, 0:LX])
                xn = xpool.tile([128, 4, D], fp8, tag="xn", name=f"xn{j}_{st}")
                for ut in range(UT):
                    nc.sync.dma_start(
                        out=xn[:, ut, :],
                        in_=xn_d[st].bitcast(fp8)[ut * 128:(ut + 1) * 128, j, :])
                XT[st], XN[st] = xt, xn

            def gates_act(psum_ap, out_ap, scale, bias_t, bcol0, nsl):
                """psum -> bf16 gates; merged unless per-slice biases needed."""
                if with_bias:
                    for m in range(nsl):
                        nc.scalar.activation(out_ap[:, m, 0:L],
                                             psum_ap[:, m, 0:L], AF.Tanh,
                                             scale=scale,
                                             bias=bias_t[:, bcol0 + m:bcol0 + m + 1])
                else:
                    nc.scalar.activation(out_ap[:, 0:nsl, 0:L],
                                         psum_ap[:, 0:nsl, 0:L], AF.Tanh,
                                         scale=scale)

            # ---- step-1 gates (i,g,o) ---------------------------------
            gig_, go_ = {}, {}
            for st in (0, 1):
                w = W[st]
                pgA = gp.tile([128, 4, T_MAX], f32, tag="pg", name=f"pgA{j}_{st}")
                for m in range(4):
                    mm(pgA[:, m, 0:L], w["we"][:, :, m * 128:(m + 1) * 128],
                       XT[st][:, :, 0:L], True, True)
                gig = work.tile([128, 4, T_MAX], bf16, tag="gig", name=f"gig{j}_{st}")
                gates_act(pgA, gig, 0.5 / ws_e[st], w["b1"], 0, 4)
                pgB = gp.tile([128, 4, T_MAX], f32, tag="pg", name=f"pgB{j}_{st}")
                for m in (4, 5):
                    mm(pgB[:, m - 4, 0:L], w["we"][:, :, m * 128:(m + 1) * 128],
                       XT[st][:, :, 0:L], True, True)
                go = work.tile([128, 2, T_MAX], bf16, tag="go", name=f"go{j}_{st}")
                gates_act(pgB, go, 0.5 / ws_e[st], w["b1"], 4, 2)
                gig_[st], go_[st] = gig, go

            # ---- step-1 cell ------------------------------------------
            cs1_, hs1_ = {}, {}
            for st in (0, 1):
                cs1 = work.tile([128, 2, T_MAX], bf16, tag="cs", bufs=4,
                                name=f"cs1{j}_{st}")
                nc.vector.scalar_tensor_tensor(
                    cs1[:, :, 0:L], gig_[st][:, 0:2, 0:L], 1.0,
                    gig_[st][:, 2:4, 0:L], ALU.add, ALU.mult)
                cs1_[st] = cs1
            th1_ = {}
            for st in (0, 1):
                th1 = work.tile([128, 2, T_MAX], bf16, tag="th", name=f"th1{j}_{st}")
                nc.scalar.activation(th1[:, :, 0:L], cs1_[st][:, :, 0:L],
                                     AF.Tanh, scale=0.5)
                th1_[st] = th1
            for st in (0, 1):
                hs1 = work.tile([128, 2, T_MAX], fp8, tag="hs", bufs=6,
                                name=f"hs1{j}_{st}")
                nc.vector.scalar_tensor_tensor(
                    hs1[:, :, 0:L], go_[st][:, :, 0:L], 1.0,
                    th1_[st][:, :, 0:L], ALU.add, ALU.mult)
                hs1_[st] = hs1

            def attention(st, hs_tile, step):
                """A = exp(0.5*e + mask); Z = 1/colsum(A). Returns (A, Z)."""
                xt = XT[st]
                A = work.tile([128, 4, T_MAX], fp8, tag="A", bufs=4,
                              name=f"A{j}_{st}_{step}")
                nfull = UT - 1
                done = 0
                while done < nfull:  # unmasked full blocks (pairs when possible)
                    take = 2 if nfull - done >= 2 else 1
                    et = ep.tile([128, 2, T_MAX], f32, tag="e",
                                 name=f"e{j}_{st}_{step}_{done}")
                    for q in range(take):
                        ut = done + q
                        mm(et[:, q, 0:L], xt[:, :, ut * 128:(ut + 1) * 128],
                           hs_tile[:, :, 0:L], True, True)
                    nc.scalar.activation(A[:, done:done + take, 0:L],
                                         et[:, 0:take, 0:L], AF.Exp, scale=0.5)
                    done += take
                et = ep.tile([128, 2, T_MAX], f32, tag="e",
                             name=f"em{j}_{st}_{step}")
                ut = UT - 1
                mm(et[:, 0, 0:L], xt[:, :, ut * 128:(ut + 1) * 128],
                   hs_tile[:, :, 0:L], True, True)
                col = j * 4 + ut
                nc.scalar.activation(A[:, ut, 0:L], et[:, 0, 0:L], AF.Exp,
                                     scale=0.5, bias=mask_t[:, col:col + 1])
                # column sums (replicated to all partitions) + reciprocal
                pt = ep.tile([128, 2, T_MAX], f32, tag="e",
                             name=f"psm{j}_{st}_{step}")
                for k in range(NPAIR):
                    mm(pt[:, 0, 0:L], ones8[:, :, :], A[:, 2 * k:2 * k + 2, 0:L],
                       k == 0, k == NPAIR - 1 and not ODD)
                if ODD:
                    mm(pt[:, 0, 0:L], ones8[:, 0, :], A[:, UT - 1, 0:L],
                       NPAIR == 0, True, pm=None)
                Z = work.tile([128, T_MAX], f32, tag="Z", bufs=3,
                              name=f"Z{j}_{st}_{step}")
                nc.vector.reciprocal_approx_fast(Z[:, 0:L], pt[:, 0, 0:L])
                if dbg:
                    nc.sync.dma_start(out=dbg_A[j, st, step - 1, :, :, 0:L],
                                      in_=A[:, :, 0:L].bitcast(u8))
                    nc.sync.dma_start(out=dbg_Z[j, st, step - 1, :, 0:L],
                                      in_=Z[:, 0:L])
                return A, Z

            def r_apply(st, A, Z, out_tile, zoff, relu, step):
                """out[:, zoff+dt, :] = (X^T A) * Z, optionally relu'd."""
                xn = XN[st]
                for dt in range(2):
                    rt = rp.tile([128, T_MAX], f32, tag="r",
                                 name=f"r{j}_{st}_{step}_{dt}")
                    for k in range(NPAIR):
                        mm(rt[:, 0:L],
                           xn[:, 2 * k:2 * k + 2, dt * 128:(dt + 1) * 128],
                           A[:, 2 * k:2 * k + 2, 0:L],
                           k == 0, k == NPAIR - 1 and not ODD)
                    if ODD:
                        mm(rt[:, 0:L], xn[:, UT - 1, dt * 128:(dt + 1) * 128],
                           A[:, UT - 1, 0:L], NPAIR == 0, True, pm=None)
                    # (GPSIMD cannot read PSUM -> both variants on DVE)
                    if relu:
                        nc.vector.scalar_tensor_tensor(
                            out_tile[:, zoff + dt, 0:L], rt[:, 0:L], 0.0,
                            Z[:, 0:L], ALU.max, ALU.mult)
                    else:
                        nc.vector.scalar_tensor_tensor(
                            out_tile[:, zoff + dt, 0:L], rt[:, 0:L], 1.0,
                            Z[:, 0:L], ALU.mult, ALU.mult)

            # ---- step-1 attention -------------------------------------
            AZ1 = {}
            for st in (0, 1):
                AZ1[st] = attention(st, hs1_[st], 1)
            r1_ = {}
            for st in (0, 1):
                r1 = work.tile([128, 2, T_MAX], fp8, tag="r1", bufs=3,
                               name=f"r1{j}_{st}")
                r_apply(st, AZ1[st][0], AZ1[st][1], r1, 0, False, 1)
                r1_[st] = r1

            # ---- step-2 gates (i,f | g,o) -----------------------------
            gif_, ggo_ = {}, {}
            for st in (0, 1):
                w = W[st]
                pgC = gp.tile([128, 4, T_MAX], f32, tag="pg", name=f"pgC{j}_{st}")
                for m in range(4):
                    mm(pgC[:, m, 0:L], w["wh"][:, :, m * 128:(m + 1) * 128],
                       hs1_[st][:, :, 0:L], True, False)
                    mm(pgC[:, m, 0:L], w["wr"][:, :, m * 128:(m + 1) * 128],
                       r1_[st][:, :, 0:L], False, True)
                gif = work.tile([128, 4, T_MAX], bf16, tag="gif", name=f"gif{j}_{st}")
                gates_act(pgC, gif, 0.5 / ws_h[st], w["b2"], 0, 4)
                pgD = gp.tile([128, 4, T_MAX], f32, tag="pg", name=f"pgD{j}_{st}")
                for m in range(4, 8):
                    mm(pgD[:, m - 4, 0:L], w["wh"][:, :, m * 128:(m + 1) * 128],
                       hs1_[st][:, :, 0:L], True, False)
                    mm(pgD[:, m - 4, 0:L], w["wr"][:, :, m * 128:(m + 1) * 128],
                       r1_[st][:, :, 0:L], False, True)
                ggo = work.tile([128, 4, T_MAX], bf16, tag="ggo", name=f"ggo{j}_{st}")
                gates_act(pgD, ggo, 0.5 / ws_h[st], w["b2"], 4, 4)
                gif_[st], ggo_[st] = gif, ggo

            # ---- step-2 cell ------------------------------------------
            cs2_ = {}
            for st in (0, 1):
                t1 = work.tile([128, 2, T_MAX], bf16, tag="tmp", bufs=4,
                               name=f"t1{j}_{st}")
                nc.vector.scalar_tensor_tensor(
                    t1[:, :, 0:L], gif_[st][:, 2:4, 0:L], 1.0,
                    cs1_[st][:, :, 0:L], ALU.add, ALU.mult)
                t2 = work.tile([128, 2, T_MAX], bf16, tag="tmp", bufs=4,
                               name=f"t2{j}_{st}")
                nc.vector.scalar_tensor_tensor(
                    t2[:, :, 0:L], gif_[st][:, 0:2, 0:L], 1.0,
                    ggo_[st][:, 0:2, 0:L], ALU.add, ALU.mult)
                cs2 = work.tile([128, 2, T_MAX], bf16, tag="cs", bufs=4,
                                name=f"cs2{j}_{st}")
                nc.vector.scalar_tensor_tensor(
                    cs2[:, :, 0:L], t1[:, :, 0:L], 0.5, t2[:, :, 0:L],
                    ALU.mult, ALU.add)
                cs2_[st] = cs2
            th2_ = {}
            for st in (0, 1):
                th2 = work.tile([128, 2, T_MAX], bf16, tag="th", name=f"th2{j}_{st}")
                nc.scalar.activation(th2[:, :, 0:L], cs2_[st][:, :, 0:L],
                                     AF.Tanh, scale=0.5)
                th2_[st] = th2
            hs2_, ft_ = {}, {}
            for st in (0, 1):
                hs2 = work.tile([128, 2, T_MAX], fp8, tag="hs", bufs=6,
                                name=f"hs2{j}_{st}")
                nc.vector.scalar_tensor_tensor(
                    hs2[:, :, 0:L], ggo_[st][:, 2:4, 0:L], 1.0,
                    th2_[st][:, :, 0:L], ALU.add, ALU.mult)
                hs2_[st] = hs2
                if dbg:
                    nc.sync.dma_start(out=dbg_hs[j, st, 1, :, :, 0:L],
                                      in_=hs2[:, :, 0:L].bitcast(u8))
                    nc.sync.dma_start(out=dbg_hs[j, st, 0, :, :, 0:L],
                                      in_=hs1_[st][:, :, 0:L].bitcast(u8))

            # ---- step-2 attention + features --------------------------
            for st in (0, 1):
                A2, Z2 = attention(st, hs2_[st], 2)
                ft = fpool.tile([128, 4, T_MAX], fp8, tag=f"ft{st}",
                                name=f"ft{j}_{st}")
                nc.gpsimd.tensor_scalar_max(ft[:, 0:2, 0:L],
                                            hs2_[st][:, :, 0:L], 0.0)
                r_apply(st, A2, Z2, ft, 2, True, 2)
                ft_[st] = ft
                if dbg:
                    nc.sync.dma_start(out=dbg_ft[j, st, :, :, 0:L],
                                      in_=ft[:, :, 0:L].bitcast(u8))

            # ---- logits (transposed: [t, C]) + exp/row-sum ------------
            lpt = lp.tile([128, 4, C], f32, tag="lg", name=f"lp{j}")
            for cch in range(UT):
                for m, (ftile, zz) in enumerate(
                        ((ft_[0], 0), (ft_[0], 2), (ft_[1], 0), (ft_[1], 2))):
                    mm(lpt[:, cch, :],
                       ftile[:, zz:zz + 2, cch * 128:(cch + 1) * 128],
                       outw_t[:, 2 * m:2 * m + 2, :], m == 0, m == 3)
            nc.scalar.activation(lg_all[:, j, 0:UT, :], lpt[:, 0:UT, :],
                                 AF.Copy, scale=1.0 / ows)
            elg = work.tile([128, 4, C], f32, tag="elg", name=f"elg{j}")
            nc.scalar.activation(elg[:, 0:UT, :], lpt[:, 0:UT, :], AF.Exp,
                                 scale=1.0 / ows)
            nc.vector.tensor_reduce(s_all[:, j * 4:j * 4 + UT],
                                    elg[:, 0:UT, :], AX.X, ALU.add)

        # ---- final: logp = lg - ln(rowsum) ----------------------------
        if dbg:
            nc.sync.dma_start(out=dbg_lg, in_=lg_all)
            nc.sync.dma_start(out=dbg_s, in_=s_all)
        nc.scalar.activation(lnS[:, :], s_all[:, :], AF.Ln)
        for j in range(NCONV):
            UT = (int(slot_lens[j]) + 127) // 128
            ot = opool.tile([128, 4, C], f32, tag="ot", name=f"ot{j}")
            for cch in range(UT):
                nc.vector.tensor_scalar_sub(ot[:, cch, :], lg_all[:, j, cch, :],
                                            lnS[:, j * 4 + cch:j * 4 + cch + 1])
            nc.sync.dma_start(
                out=out_d[j].rearrange("(c p) k -> p c k", p=128)[:, 0:UT, :],
                in_=ot[:, 0:UT, :])

    nc.compile()
    return nc


def _host_prep(inputs):
    """Fold weights, quantize to fp8, pick conversation->core assignment."""
    x_s = np.asarray(inputs["input"], dtype=np.float32)
    x_p = np.asarray(inputs["speakers"], dtype=np.float32)
    lengths = np.asarray(inputs["utterance_lengths"]).astype(np.int64)
    fc_w = np.asarray(inputs["fc_w"], dtype=np.float32)
    fc_b = np.asarray(inputs["fc_b"], dtype=np.float32)
    out_w = np.asarray(inputs["out_w"], dtype=np.float32)
    out_b = np.asarray(inputs["out_b"], dtype=np.float32)

    per_stream = {}
    scales = {}
    any_b = bool(np.any(out_b != 0.0))
    for st in ("s", "p"):
        w_ih = np.asarray(inputs[f"w_ih_{st}"], dtype=np.float32)
        w_hh = np.asarray(inputs[f"w_hh_{st}"], dtype=np.float32)
        b_ih = np.asarray(inputs[f"b_ih_{st}"], dtype=np.float32)
        b_hh = np.asarray(inputs[f"b_hh_{st}"], dtype=np.float32)
        W_eff = w_ih @ fc_w                          # [1024, 256]
        bias1 = w_ih @ fc_b + b_ih + b_hh            # [1024]
        sel = np.r_[0:D, 2 * D:4 * D]                # i, g, o rows
        We = np.ascontiguousarray(W_eff[sel].T)      # [256, 768]
        We[:, D:2 * D] *= 2.0                        # g-gate doubling
        Wh = np.ascontiguousarray((0.5 * (w_ih[:, :D] + w_hh)).T)  # [256, 1024]
        Wr = np.ascontiguousarray(w_ih[:, D:].T)     # [256, 1024]
        Wh[:, 2 * D:3 * D] *= 2.0
        Wr[:, 2 * D:3 * D] *= 2.0
        ws_e = _pow2_scale(We)
        ws_h = _pow2_scale(np.concatenate([Wh, Wr], axis=0))
        scales[f'ws_e_{st}'] = ws_e
        scales[f'ws_h_{st}'] = ws_h
        # per-slice activation biases ([128, nslices], pre-multiplied by the
        # tanh input scale: 0.5 normally, 1.0 for the doubled g-gate)
        b1_sel = bias1[sel]                          # [768] i,g,o
        bias2 = b_ih + b_hh                          # [1024] i,f,g,o
        b1_cols = np.zeros((128, 6), np.float32)
        for m in range(6):
            f = 1.0 if m in (2, 3) else 0.5
            b1_cols[:, m] = f * b1_sel[m * 128:(m + 1) * 128]
        b2_cols = np.zeros((128, 8), np.float32)
        for m in range(8):
            f = 1.0 if m in (4, 5) else 0.5
            b2_cols[:, m] = f * bias2[m * 128:(m + 1) * 128]
        any_b |= bool(np.any(b1_cols != 0.0) or np.any(b2_cols != 0.0))
        per_stream[st] = (_f8(We * ws_e), _f8(Wh * ws_h), _f8(Wr * ws_h),
                          b1_cols, b2_cols)

    # out_w columns for the h-halves get the 0.5 compensation (h stored as 2h)
    ow = out_w.copy()
    ow[:, 0:D] *= 0.5
    ow[:, 2 * D:3 * D] *= 0.5
    ows = _pow2_scale(ow)
    scales['ows'] = ows
    outw8 = _f8(ow.T * ows)                          # [1024, 7]
    # out_b support: fold into lg via... biases are zero in practice; if
    # nonzero we add them on the host after gather (logp is shift-invariant
    # per row only for the ln term; lg needs +b before softmax) -> handled
    # by host_logits_bias below.
    host_out_b = out_b

    # conversation -> (core, slot): sort by length desc, round-robin
    order = np.argsort(-lengths, kind="stable")
    assign = {}
    for rank, conv in enumerate(order):
        assign[int(conv)] = (rank % NCORE, rank // NCORE)
    order_lens = lengths[order]
    slot_lens = tuple(int(order_lens[8 * k]) for k in range(NCONV))

    # fp8-quantize the banks once (identical bytes for both layouts)
    import ml_dtypes
    xs8 = np.clip(x_s, -240.0, 240.0).astype(ml_dtypes.float8_e4m3fn).view(np.uint8)
    xp8 = np.clip(x_p, -240.0, 240.0).astype(ml_dtypes.float8_e4m3fn).view(np.uint8)

    in_maps = []
    core_convs = []
    for core in range(NCORE):
        ids = [None] * NCONV
        for conv, (c, s) in assign.items():
            if c == core:
                ids[s] = conv
        core_convs.append(ids)
        mask = np.zeros((128, NCONV * 4), dtype=np.float32)
        for s, conv in enumerate(ids):
            Lc = int(lengths[conv])
            u = np.arange(T_MAX)
            m = np.where(u < Lc, 0.0, MASKV).astype(np.float32)
            mask[:, s * 4:(s + 1) * 4] = m.reshape(4, 128).T
        im = {
            "xts": np.ascontiguousarray(
                xs8[:, ids, :].transpose(1, 2, 0).reshape(NCONV, 2, 128, T_MAX)),
            "xtp": np.ascontiguousarray(
                xp8[:, ids, :].transpose(1, 2, 0).reshape(NCONV, 2, 128, T_MAX)),
            "xns": np.ascontiguousarray(xs8[:, ids, :]),
            "xnp": np.ascontiguousarray(xp8[:, ids, :]),
            "mask": mask,
            "ones8": _f8(np.ones((128, 256), np.float32)),
            "outw": outw8,
        }
        for st in ("s", "p"):
            We8, Wh8, Wr8, b1c, b2c = per_stream[st]
            im[f"we_{st}"] = We8
            im[f"wh_{st}"] = Wh8
            im[f"wr_{st}"] = Wr8
            if any_b:
                im[f"b1_{st}"] = b1c
                im[f"b2_{st}"] = b2c
        in_maps.append(im)
    key = (any_b, slot_lens,
           tuple(sorted((k, float(v)) for k, v in scales.items())))
    return in_maps, core_convs, lengths, key, scales, host_out_b


def _gather(results, core_convs, lengths, out_b):
    """results: per-core {'out': [NCONV, T_MAX, C]} -> [sum(len), C]."""
    where = {}
    for core, ids in enumerate(core_convs):
        for slot, conv in enumerate(ids):
            where[conv] = (core, slot)
    chunks = []
    nz = bool(np.any(out_b != 0.0))
    for b in range(BATCH):
        core, slot = where[b]
        L = int(lengths[b])
        lp = results[core]["out"][slot, :L, :]
        if nz:
            # device computed log-softmax without out_b; redo it exactly
            lg = lp + 0.0  # lp = lg - ln(sum(exp(lg))); recover lg shift-free
            lg = lg + out_b[None, :]
            lp = lg - np.log(np.exp(lg).sum(axis=1, keepdims=True))
        chunks.append(np.ascontiguousarray(lp))
    return np.concatenate(chunks, axis=0).astype(np.float32)


def _get_nc(key, scales):
    if key not in _BUILD_CACHE:
        _BUILD_CACHE[key] = _build(key[0], key[1], scales)
    return _BUILD_CACHE[key]


def kernel(**inputs):
    from concourse import bass_utils
    in_maps, core_convs, lengths, key, scales, out_b = _host_prep(inputs)
    nc = _get_nc(key, scales)
    res = bass_utils.run_bass_kernel_spmd(nc, in_maps, core_ids=list(range(NCORE)))
    return _gather(res.results, core_convs, lengths, out_b)


# revision 13
# speedup vs baseline: 1.4943x; 1.4321x over previous
"""DCRNCognition Trainium2 kernel — fp8 DoubleRow edition.

Self-contained: builds a Bass/Tile SPMD program for 8 NeuronCores, shards the
batch (conversation) axis across cores, runs via run_bass_kernel_spmd, and
gathers the valid positions on the host.

Math restructuring (identical to the verified baseline, rel err ~9e-7 in f32):
  - fc layer folded into step-1 LSTM gates; step-1 f-gate/c-init dead
  - step-2: gates2 = hs1 @ Wh.T + r1 @ Wr.T  (Wh = 0.5*(w_ih[:, :D]+w_hh))
  - softmax normalization deferred to r:  r = (X^T A) * (1/sum_u A)
  - sigmoid via tanh; h,c carried scaled by 2 (hs=2h, cs=2c)

Precision plan (validated on host: rel err ~6e-4 vs the 2e-2 gate):
  - ALL matmuls in fp8 e4m3 with DoubleRow perf mode, fp32 PSUM accum.
  - weights pre-scaled by a power of 2 into fp8 range on the host; the
    compensation folds into the free activation `scale` operands.
  - g-gate weight rows doubled on host so every gate activation shares
    scale=0.5 -> gate groups convert with one ACT instruction per gate.
  - element-wise intermediates bf16 on exact-size (contiguous) tiles.
  - 1/sum via reciprocal_approx_fast (DVE custom op).
  - logits computed transposed ([t,C]): log-softmax reduces along the free
    axis; exp per conv in the main loop (same ACT table as tanh), one
    batched Ln at the end -> 2 table loads total.

Scheduling: software-pipelined with a 1-conversation skew -- front(j) =
DMA+gates1+cell1 is emitted before back(j-1) = attention/gates2/logits so
the in-order PE queue always holds independent work.
"""
import os
import sys
sys.path.insert(0, '/opt/trn_rl_repo')

# run_bass_kernel_spmd executes through jax/PJRT on the axon-tunneled
# NeuronCores; a JAX_PLATFORMS=cpu pin would hide them.
if os.environ.get('JAX_PLATFORMS') == 'cpu' and 'jax' not in sys.modules:
    del os.environ['JAX_PLATFORMS']

import math
import numpy as np

T_MAX, BATCH, D, C = 512, 128, 256, 7
NCORE = 8
NCONV = BATCH // NCORE          # conversations per core
MASKV = -30000.0                # additive pre-exp mask for invalid bank rows

_BUILD_CACHE = {}


def _f8(x):
    """Host fp32 -> e4m3 bytes (clipped to the TRN-compatible +-240 range)."""
    import ml_dtypes
    return np.ascontiguousarray(
        np.clip(np.asarray(x, np.float32), -240.0, 240.0)
        .astype(ml_dtypes.float8_e4m3fn).view(np.uint8))


def _pow2_scale(w):
    s = float(np.std(w))
    if s == 0.0 or not np.isfinite(s):
        return 1.0
    return float(2.0 ** round(math.log2(4.0 / s)))


def _build(with_bias, slot_lens, scales):
    """Build + compile the SPMD Bass program. Returns the Bacc instance."""
    from contextlib import ExitStack
    import concourse.bacc as bacc
    import concourse.bass as bass  # noqa: F401
    from concourse import mybir, tile

    f32 = mybir.dt.float32
    bf16 = mybir.dt.bfloat16
    fp8 = mybir.dt.float8e4
    u8 = mybir.dt.uint8
    AF = mybir.ActivationFunctionType
    ALU = mybir.AluOpType
    AX = mybir.AxisListType
    PM = mybir.MatmulPerfMode.DoubleRow

    ws_e = {0: scales['ws_e_s'], 1: scales['ws_e_p']}
    ws_h = {0: scales['ws_h_s'], 1: scales['ws_h_p']}
    ows = scales['ows']

    nc = bacc.Bacc("TRN2", target_bir_lowering=False, debug=False,
                   num_devices=NCORE)

    def din(name, shape, dt):
        return nc.dram_tensor(name, shape, dt, kind="ExternalInput").ap()

    xt_d = {0: din("xts", [NCONV, 2, 128, T_MAX], u8),
            1: din("xtp", [NCONV, 2, 128, T_MAX], u8)}
    xn_d = {0: din("xns", [T_MAX, NCONV, D], u8),
            1: din("xnp", [T_MAX, NCONV, D], u8)}
    wdefs = {}
    for sti, st in enumerate(("s", "p")):
        wdefs[sti] = dict(
            we=din(f"we_{st}", [D, 768], u8),
            wh=din(f"wh_{st}", [D, 1024], u8),
            wr=din(f"wr_{st}", [D, 1024], u8),
            b1=din(f"b1_{st}", [128, 6], f32) if with_bias else None,
            b2=din(f"b2_{st}", [128, 8], f32) if with_bias else None,
        )
    ones_d = din("ones8", [128, 256], u8)
    mask_d = din("mask", [128, NCONV * 4], f32)
    outw_d = din("outw", [4 * D, C], u8)
    out_d = nc.dram_tensor("out", [NCONV, T_MAX, C], f32,
                           kind="ExternalOutput").ap()

    with ExitStack() as ctx:
        tc = ctx.enter_context(tile.TileContext(nc))
        const = ctx.enter_context(tc.tile_pool(name="const", bufs=1))
        xpool = ctx.enter_context(tc.tile_pool(name="xpool", bufs=4))
        work = ctx.enter_context(tc.tile_pool(name="work", bufs=2))
        fpool = ctx.enter_context(tc.tile_pool(name="fpool", bufs=2))
        opool = ctx.enter_context(tc.tile_pool(name="opool", bufs=2))
        gp = ctx.enter_context(tc.tile_pool(name="gp", bufs=2, space="PSUM"))
        ep = ctx.enter_context(tc.tile_pool(name="ep", bufs=1, space="PSUM"))
        rp = ctx.enter_context(tc.tile_pool(name="rp", bufs=1, space="PSUM"))
        lp = ctx.enter_context(tc.tile_pool(name="lp", bufs=1, space="PSUM"))

        # ---- constants / weights --------------------------------------
        W = {}
        for sti, st in enumerate(("s", "p")):
            d = wdefs[sti]
            we_t = const.tile([128, 2, 768], fp8, name=f"we_t{st}")
            nc.sync.dma_start(out=we_t, in_=d["we"].bitcast(fp8).rearrange(
                "(kt p) m -> p kt m", p=128))
            wh_t = const.tile([128, 2, 1024], fp8, name=f"wh_t{st}")
            nc.sync.dma_start(out=wh_t, in_=d["wh"].bitcast(fp8).rearrange(
                "(kt p) m -> p kt m", p=128))
            wr_t = const.tile([128, 2, 1024], fp8, name=f"wr_t{st}")
            nc.sync.dma_start(out=wr_t, in_=d["wr"].bitcast(fp8).rearrange(
                "(kt p) m -> p kt m", p=128))
            b1_t = b2_t = None
            if with_bias:
                b1_t = const.tile([128, 6], f32, name=f"b1_t{st}")
                nc.sync.dma_start(out=b1_t, in_=d["b1"])
                b2_t = const.tile([128, 8], f32, name=f"b2_t{st}")
                nc.sync.dma_start(out=b2_t, in_=d["b2"])
            W[sti] = dict(we=we_t, wh=wh_t, wr=wr_t, b1=b1_t, b2=b2_t)
        ones8 = const.tile([128, 2, 128], fp8, name="ones8")
        nc.sync.dma_start(out=ones8, in_=ones_d.bitcast(fp8).rearrange(
            "p (a b) -> p a b", a=2))
        mask_t = const.tile([128, NCONV * 4], f32, name="mask_t")
        nc.sync.dma_start(out=mask_t, in_=mask_d)
        outw_t = const.tile([128, 8, C], fp8, name="outw_t")
        nc.sync.dma_start(out=outw_t, in_=outw_d.bitcast(fp8).rearrange(
            "(kt p) c -> p kt c", p=128))
        lg_all = const.tile([128, NCONV, 4, C], f32, name="lg_all")
        s_all = const.tile([128, NCONV * 4], f32, name="s_all")
        lnS = const.tile([128, NCONV * 4], f32, name="lnS")

        def mm(ps, lhsT, rhs, start, stop, pm=PM):
            nc.tensor.matmul(ps, lhsT, rhs, start=start, stop=stop,
                             perf_mode=pm)

        def dims(j):
            Lv = int(slot_lens[j])
            # 16-aligned: DoubleRow LDWEIGHTS requires k-pair step % 16 == 0
            L = min(T_MAX, ((Lv + 15) // 16) * 16)
            UT = (Lv + 127) // 128
            return Lv, L, UT, UT * 128

        S = [None] * NCONV      # per-conv pipeline state

        def gate_act(pg_t, out_t, L, scale, bias_t, bcol):
            """[128,2,L] psum -> bf16, merged unless per-z biases needed."""
            if with_bias:
                for z in range(2):
                    nc.scalar.activation(out_t[:, z, :], pg_t[:, z, 0:L],
                                         AF.Tanh, scale=scale,
                                         bias=bias_t[:, bcol + z:bcol + z + 1])
            else:
                nc.scalar.activation(out_t[:, :, :], pg_t[:, :, 0:L],
                                     AF.Tanh, scale=scale)

        def emit_front(j):
            Lv, L, UT, LX = dims(j)
            xt_, xn_, g1_ = {}, {}, {}
            for st in (0, 1):
                xt = xpool.tile([128, 2, LX], fp8, tag="xt", name=f"xt{j}_{st}")
                for kd in range(2):
                    nc.sync.dma_start(out=xt[:, kd, :],
                                      in_=xt_d[st].bitcast(fp8)[j, kd, :, 0:LX])
                xn = xpool.tile([128, 4, D], fp8, tag="xn", name=f"xn{j}_{st}")
                for ut in range(UT):
                    nc.sync.dma_start(
                        out=xn[:, ut, :],
                        in_=xn_d[st].bitcast(fp8)[ut * 128:(ut + 1) * 128, j, :])
                xt_[st], xn_[st] = xt, xn
            for st in (0, 1):
                w = W[st]
                gts = []
                for gi in range(3):            # i, g, o
                    pg_t = gp.tile([128, 2, T_MAX], f32, tag="pg",
                                   name=f"pg1{j}_{st}_{gi}")
                    for z in range(2):
                        mm(pg_t[:, z, 0:L],
                           w["we"][:, :, (2 * gi + z) * 128:(2 * gi + z + 1) * 128],
                           xt_[st][:, :, 0:L], True, True)
                    gt = work.tile([128, 2, L], bf16, tag="g1", bufs=12,
                                   name=f"g1{j}_{st}_{gi}")
                    gate_act(pg_t, gt, L, 0.5 / ws_e[st], w["b1"], 2 * gi)
                    gts.append(gt)
                g1_[st] = gts
            cs1_, hs1_ = {}, {}
            for st in (0, 1):
                gi, gg, go = g1_[st]
                cs1 = work.tile([128, 2, L], bf16, tag="cs", bufs=8,
                                name=f"cs1{j}_{st}")
                nc.vector.scalar_tensor_tensor(cs1[:, :, :], gi[:, :, :], 1.0,
                                               gg[:, :, :], ALU.add, ALU.mult)
                cs1_[st] = cs1
            th1_ = {}
            for st in (0, 1):
                th1 = work.tile([128, 2, L], bf16, tag="th", bufs=4,
                                name=f"th1{j}_{st}")
                nc.scalar.activation(th1[:, :, :], cs1_[st][:, :, :],
                                     AF.Tanh, scale=0.5)
                th1_[st] = th1
            for st in (0, 1):
                hs1 = work.tile([128, 2, L], fp8, tag="hs", bufs=8,
                                name=f"hs1{j}_{st}")
                nc.vector.scalar_tensor_tensor(hs1[:, :, :],
                                               g1_[st][2][:, :, :], 1.0,
                                               th1_[st][:, :, :],
                                               ALU.add, ALU.mult)
                hs1_[st] = hs1
            S[j] = dict(xt=xt_, xn=xn_, cs1=cs1_, hs1=hs1_)

        def attention(j, st, hs_tile, step, L, UT):
            """A = exp(0.5*e + mask); Z = 1/colsum(A)."""
            xt = S[j]['xt'][st]
            A = work.tile([128, 4, L], fp8, tag="A", bufs=4,
                          name=f"A{j}_{st}_{step}")
            nfull, done = UT - 1, 0
            while done < nfull:
                take = 2 if nfull - done >= 2 else 1
                et = ep.tile([128, 2, T_MAX], f32, tag="e",
                             name=f"e{j}_{st}_{step}_{done}")
                for q in range(take):
                    ut = done + q
                    mm(et[:, q, 0:L], xt[:, :, ut * 128:(ut + 1) * 128],
                       hs_tile[:, :, :], True, True)
                nc.scalar.activation(A[:, done:done + take, :],
                                     et[:, 0:take, 0:L], AF.Exp, scale=0.5)
                done += take
            et = ep.tile([128, 2, T_MAX], f32, tag="e", name=f"em{j}_{st}_{step}")
            ut = UT - 1
            mm(et[:, 0, 0:L], xt[:, :, ut * 128:(ut + 1) * 128],
               hs_tile[:, :, :], True, True)
            col = j * 4 + ut
            nc.scalar.activation(A[:, ut, :], et[:, 0, 0:L], AF.Exp,
                                 scale=0.5, bias=mask_t[:, col:col + 1])
            NPAIR, ODD = UT // 2, UT % 2
            pt = ep.tile([128, 2, T_MAX], f32, tag="e", name=f"ps{j}_{st}_{step}")
            for k in range(NPAIR):
                mm(pt[:, 0, 0:L], ones8[:, :, :], A[:, 2 * k:2 * k + 2, :],
                   k == 0, k == NPAIR - 1 and not ODD)
            if ODD:
                mm(pt[:, 0, 0:L], ones8[:, 0, :], A[:, UT - 1, :],
                   NPAIR == 0, True, pm=None)
            Z = work.tile([128, L], f32, tag="Z", bufs=4, name=f"Z{j}_{st}_{step}")
            nc.vector.reciprocal_approx_fast(Z[:, :], pt[:, 0, 0:L])
            return A, Z

        def r_apply(j, st, A, Z, out_tile, zoff, relu, step, L, UT):
            """out[:, zoff+dt, :] = (X^T A) * Z, optionally relu'd."""
            xn = S[j]['xn'][st]
            NPAIR, ODD = UT // 2, UT % 2
            for dt in range(2):
                rt = rp.tile([128, T_MAX], f32, tag="r",
                             name=f"r{j}_{st}_{step}_{dt}")
                for k in range(NPAIR):
                    mm(rt[:, 0:L],
                       xn[:, 2 * k:2 * k + 2, dt * 128:(dt + 1) * 128],
                       A[:, 2 * k:2 * k + 2, :], k == 0,
                       k == NPAIR - 1 and not ODD)
                if ODD:
                    mm(rt[:, 0:L], xn[:, UT - 1, dt * 128:(dt + 1) * 128],
                       A[:, UT - 1, :], NPAIR == 0, True, pm=None)
                nc.vector.scalar_tensor_tensor(
                    out_tile[:, zoff + dt, :], rt[:, 0:L],
                    0.0 if relu else 1.0, Z[:, :],
                    ALU.max if relu else ALU.mult, ALU.mult)

        def emit_back(j):
            Lv, L, UT, LX = dims(j)
            st_ = S[j]
            AZ1, r1_ = {}, {}
            for st in (0, 1):
                AZ1[st] = attention(j, st, st_['hs1'][st], 1, L, UT)
            for st in (0, 1):
                r1 = work.tile([128, 2, L], fp8, tag="r1", bufs=4,
                               name=f"r1{j}_{st}")
                r_apply(j, st, AZ1[st][0], AZ1[st][1], r1, 0, False, 1, L, UT)
                r1_[st] = r1
            g2_ = {}
            for st in (0, 1):
                w = W[st]
                gts = []
                for gi in range(4):            # i, f, g, o
                    pg_t = gp.tile([128, 2, T_MAX], f32, tag="pg",
                                   name=f"pg2{j}_{st}_{gi}")
                    for z in range(2):
                        m = 2 * gi + z
                        mm(pg_t[:, z, 0:L],
                           w["wh"][:, :, m * 128:(m + 1) * 128],
                           st_['hs1'][st][:, :, :], True, False)
                        mm(pg_t[:, z, 0:L],
                           w["wr"][:, :, m * 128:(m + 1) * 128],
                           r1_[st][:, :, :], False, True)
                    gt = work.tile([128, 2, L], bf16, tag="g2", bufs=10,
                                   name=f"g2{j}_{st}_{gi}")
                    gate_act(pg_t, gt, L, 0.5 / ws_h[st], w["b2"], 2 * gi)
                    gts.append(gt)
                g2_[st] = gts
            cs2_ = {}
            for st in (0, 1):
                gi2, gf2, gg2, go2 = g2_[st]
                t1 = work.tile([128, 2, L], bf16, tag="tmp", bufs=4,
                               name=f"t1{j}_{st}")
                nc.vector.scalar_tensor_tensor(t1[:, :, :], gf2[:, :, :], 1.0,
                                               st_['cs1'][st][:, :, :],
                                               ALU.add, ALU.mult)
                t2 = work.tile([128, 2, L], bf16, tag="tmp", bufs=4,
                               name=f"t2{j}_{st}")
                nc.vector.scalar_tensor_tensor(t2[:, :, :], gi2[:, :, :], 1.0,
                                               gg2[:, :, :], ALU.add, ALU.mult)
                cs2 = work.tile([128, 2, L], bf16, tag="cs", bufs=8,
                                name=f"cs2{j}_{st}")
                nc.vector.scalar_tensor_tensor(cs2[:, :, :], t1[:, :, :], 0.5,
                                               t2[:, :, :], ALU.mult, ALU.add)
                cs2_[st] = cs2
            th2_ = {}
            for st in (0, 1):
                th2 = work.tile([128, 2, L], bf16, tag="th", bufs=4,
                                name=f"th2{j}_{st}")
                nc.scalar.activation(th2[:, :, :], cs2_[st][:, :, :],
                                     AF.Tanh, scale=0.5)
                th2_[st] = th2
            hs2_ = {}
            for st in (0, 1):
                hs2 = work.tile([128, 2, L], fp8, tag="hs", bufs=8,
                                name=f"hs2{j}_{st}")
                nc.vector.scalar_tensor_tensor(hs2[:, :, :],
                                               g2_[st][3][:, :, :], 1.0,
                                               th2_[st][:, :, :],
                                               ALU.add, ALU.mult)
                hs2_[st] = hs2
            ft_ = {}
            for st in (0, 1):
                A2, Z2 = attention(j, st, hs2_[st], 2, L, UT)
                ft = fpool.tile([128, 4, L], fp8, tag=f"ft{st}", name=f"ft{j}_{st}")
                nc.vector.tensor_scalar_max(ft[:, 0:2, :], hs2_[st][:, :, :], 0.0)
                r_apply(j, st, A2, Z2, ft, 2, True, 2, L, UT)
                ft_[st] = ft
            # logits transposed: [t, C] per 128-t chunk
            lpt = lp.tile([128, 4, C], f32, tag="lg", name=f"lp{j}")
            for cch in range(UT):
                ncch = min(128, L - cch * 128)
                for m, (ftile, zz) in enumerate(
                        ((ft_[0], 0), (ft_[0], 2), (ft_[1], 0), (ft_[1], 2))):
                    mm(lpt[0:ncch, cch, :],
                       ftile[:, zz:zz + 2, cch * 128:cch * 128 + ncch],
                       outw_t[:, 2 * m:2 * m + 2, :], m == 0, m == 3)
            nc.scalar.activation(lg_all[:, j, 0:UT, :], lpt[:, 0:UT, :],
                                 AF.Copy, scale=1.0 / ows)
            elg = work.tile([128, 4, C], f32, tag="elg", name=f"elg{j}")
            nc.scalar.activation(elg[:, 0:UT, :], lpt[:, 0:UT, :], AF.Exp,
                                 scale=1.0 / ows)
            nc.vector.tensor_reduce(s_all[:, j * 4:j * 4 + UT],
                                    elg[:, 0:UT, :], AX.X, ALU.add)

        # ---- software-pipelined main loop ------------------------------
        emit_front(0)
        for j in range(1, NCONV):
            emit_front(j)
            emit_back(j - 1)
        emit_back(NCONV - 1)

        # ---- final: logp = lg - ln(rowsum) ----------------------------
        nc.scalar.activation(lnS[:, :], s_all[:, :], AF.Ln)
        for j in range(NCONV):
            UT = dims(j)[2]
            ot = opool.tile([128, 4, C], f32, tag="ot", name=f"ot{j}")
            for cch in range(UT):
                nc.vector.tensor_scalar_sub(ot[:, cch, :], lg_all[:, j, cch, :],
                                            lnS[:, j * 4 + cch:j * 4 + cch + 1])
            nc.sync.dma_start(
                out=out_d[j].rearrange("(c p) k -> p c k", p=128)[:, 0:UT, :],
                in_=ot[:, 0:UT, :])

    nc.compile()
    return nc


def _host_prep(inputs):
    """Fold weights, quantize to fp8, pick conversation->core assignment."""
    x_s = np.asarray(inputs["input"], dtype=np.float32)
    x_p = np.asarray(inputs["speakers"], dtype=np.float32)
    lengths = np.asarray(inputs["utterance_lengths"]).astype(np.int64)
    fc_w = np.asarray(inputs["fc_w"], dtype=np.float32)
    fc_b = np.asarray(inputs["fc_b"], dtype=np.float32)
    out_w = np.asarray(inputs["out_w"], dtype=np.float32)
    out_b = np.asarray(inputs["out_b"], dtype=np.float32)

    per_stream = {}
    scales = {}
    any_b = False
    for st in ("s", "p"):
        w_ih = np.asarray(inputs[f"w_ih_{st}"], dtype=np.float32)
        w_hh = np.asarray(inputs[f"w_hh_{st}"], dtype=np.float32)
        b_ih = np.asarray(inputs[f"b_ih_{st}"], dtype=np.float32)
        b_hh = np.asarray(inputs[f"b_hh_{st}"], dtype=np.float32)
        W_eff = w_ih @ fc_w                          # [1024, 256]
        bias1 = w_ih @ fc_b + b_ih + b_hh            # [1024]
        sel = np.r_[0:D, 2 * D:4 * D]                # i, g, o rows
        We = np.ascontiguousarray(W_eff[sel].T)      # [256, 768]
        We[:, D:2 * D] *= 2.0                        # g-gate doubling
        Wh = np.ascontiguousarray((0.5 * (w_ih[:, :D] + w_hh)).T)  # [256, 1024]
        Wr = np.ascontiguousarray(w_ih[:, D:].T)     # [256, 1024]
        Wh[:, 2 * D:3 * D] *= 2.0
        Wr[:, 2 * D:3 * D] *= 2.0
        ws_e = _pow2_scale(We)
        ws_h = _pow2_scale(np.concatenate([Wh, Wr], axis=0))
        scales[f'ws_e_{st}'] = ws_e
        scales[f'ws_h_{st}'] = ws_h
        # per-slice activation biases (pre-multiplied by the tanh input
        # scale: 0.5 normally, 1.0 for the doubled g-gate)
        b1_sel = bias1[sel]                          # [768] i,g,o
        bias2 = b_ih + b_hh                          # [1024] i,f,g,o
        b1_cols = np.zeros((128, 6), np.float32)
        for m in range(6):
            f = 1.0 if m in (2, 3) else 0.5
            b1_cols[:, m] = f * b1_sel[m * 128:(m + 1) * 128]
        b2_cols = np.zeros((128, 8), np.float32)
        for m in range(8):
            f = 1.0 if m in (4, 5) else 0.5
            b2_cols[:, m] = f * bias2[m * 128:(m + 1) * 128]
        any_b |= bool(np.any(b1_cols != 0.0) or np.any(b2_cols != 0.0))
        per_stream[st] = (_f8(We * ws_e), _f8(Wh * ws_h), _f8(Wr * ws_h),
                          b1_cols, b2_cols)

    # out_w columns for the h-halves get the 0.5 compensation (h stored as 2h)
    ow = out_w.copy()
    ow[:, 0:D] *= 0.5
    ow[:, 2 * D:3 * D] *= 0.5
    ows = _pow2_scale(ow)
    scales['ows'] = ows
    outw8 = _f8(ow.T * ows)                          # [1024, 7]
    host_out_b = out_b

    # conversation -> (core, slot): sort by length desc, round-robin
    order = np.argsort(-lengths, kind="stable")
    assign = {}
    for rank, conv in enumerate(order):
        assign[int(conv)] = (rank % NCORE, rank // NCORE)
    order_lens = lengths[order]
    slot_lens = tuple(int(order_lens[8 * k]) for k in range(NCONV))

    # fp8-quantize the banks once (identical bytes for both layouts)
    import ml_dtypes
    xs8 = np.clip(x_s, -240.0, 240.0).astype(ml_dtypes.float8_e4m3fn).view(np.uint8)
    xp8 = np.clip(x_p, -240.0, 240.0).astype(ml_dtypes.float8_e4m3fn).view(np.uint8)

    in_maps = []
    core_convs = []
    for core in range(NCORE):
        ids = [None] * NCONV
        for conv, (c, s) in assign.items():
            if c == core:
                ids[s] = conv
        core_convs.append(ids)
        mask = np.zeros((128, NCONV * 4), dtype=np.float32)
        for s, conv in enumerate(ids):
            Lc = int(lengths[conv])
            u = np.arange(T_MAX)
            m = np.where(u < Lc, 0.0, MASKV).astype(np.float32)
            mask[:, s * 4:(s + 1) * 4] = m.reshape(4, 128).T
        im = {
            "xts": np.ascontiguousarray(
                xs8[:, ids, :].transpose(1, 2, 0).reshape(NCONV, 2, 128, T_MAX)),
            "xtp": np.ascontiguousarray(
                xp8[:, ids, :].transpose(1, 2, 0).reshape(NCONV, 2, 128, T_MAX)),
            "xns": np.ascontiguousarray(xs8[:, ids, :]),
            "xnp": np.ascontiguousarray(xp8[:, ids, :]),
            "mask": mask,
            "ones8": _f8(np.ones((128, 256), np.float32)),
            "outw": outw8,
        }
        for st in ("s", "p"):
            We8, Wh8, Wr8, b1c, b2c = per_stream[st]
            im[f"we_{st}"] = We8
            im[f"wh_{st}"] = Wh8
            im[f"wr_{st}"] = Wr8
            if any_b:
                im[f"b1_{st}"] = b1c
                im[f"b2_{st}"] = b2c
        in_maps.append(im)
    key = (any_b, slot_lens,
           tuple(sorted((k, float(v)) for k, v in scales.items())))
    return in_maps, core_convs, lengths, key, scales, host_out_b


def _gather(results, core_convs, lengths, out_b):
    """results: per-core {'out': [NCONV, T_MAX, C]} -> [sum(len), C]."""
    where = {}
    for core, ids in enumerate(core_convs):
        for slot, conv in enumerate(ids):
            where[conv] = (core, slot)
    chunks = []
    nz = bool(np.any(out_b != 0.0))
    for b in range(BATCH):
        core, slot = where[b]
        L = int(lengths[b])
        lg = results[core]["out"][slot, :L, :]
        if nz:
            # device log-softmax omitted out_b; log_softmax is shift-invariant
            # per row, so redo it with the bias added.
            lg = lg + out_b[None, :]
            lg = lg - np.log(np.exp(lg).sum(axis=1, keepdims=True))
        chunks.append(np.ascontiguousarray(lg))
    return np.concatenate(chunks, axis=0).astype(np.float32)


def _get_nc(key, scales):
    if key not in _BUILD_CACHE:
        _BUILD_CACHE[key] = _build(key[0], key[1], scales)
    return _BUILD_CACHE[key]


def kernel(**inputs):
    from concourse import bass_utils
    in_maps, core_convs, lengths, key, scales, out_b = _host_prep(inputs)
    nc = _get_nc(key, scales)
    res = bass_utils.run_bass_kernel_spmd(nc, in_maps, core_ids=list(range(NCORE)))
    return _gather(res.results, core_convs, lengths, out_b)


# revision 15
# speedup vs baseline: 1.6936x; 1.1334x over previous
"""DCRNCognition Trainium2 kernel — fp8 DoubleRow edition.

Self-contained: builds a Bass/Tile SPMD program for 8 NeuronCores, shards the
batch (conversation) axis across cores, runs via run_bass_kernel_spmd, and
gathers the valid positions on the host.

Math restructuring (identical to the verified baseline, rel err ~9e-7 in f32):
  - fc layer folded into step-1 LSTM gates; step-1 f-gate/c-init dead
  - step-2: gates2 = hs1 @ Wh.T + r1 @ Wr.T  (Wh = 0.5*(w_ih[:, :D]+w_hh))
  - softmax normalization deferred to r:  r = (X^T A) * (1/sum_u A)
  - sigmoid via tanh; h,c carried scaled by 2 (hs=2h, cs=2c)

Precision plan (validated on host: rel err ~6e-4 vs the 2e-2 gate):
  - ALL matmuls fp8 e4m3 with DoubleRow perf mode, fp32 PSUM accum.
  - weights pre-scaled by a power of 2 into fp8 range on the host; the
    compensation folds into the free activation `scale` operands.
  - g-gate weight rows doubled on host so every gate activation shares
    scale=0.5 -> one ACT instruction per gate pair.
  - masking without exp bias: invalid bank rows (u >= len) are zeroed on
    the host in the u-major layout (kills their r contribution) and the
    softmax denominator contracts A against a per-conversation 0/1 mask
    as the matmul stationary (kills them in the sum).
  - logits computed transposed ([t,C]): log-softmax along the free axis,
    one batched Ln at the end -> 2 ACT table loads total.

Scheduling: 4-stage software pipeline (F=gates1+cell1, B1=attention1,
B2=gates2+cell2, B3=attention2+logits) with a 3-conversation skew so every
engine queue always holds ready work. PSUM: two 4-bank pools (gates /
attention+logits).
"""
import os
import sys
sys.path.insert(0, '/opt/trn_rl_repo')

# run_bass_kernel_spmd executes through jax/PJRT on the axon-tunneled
# NeuronCores; a JAX_PLATFORMS=cpu pin would hide them.
if os.environ.get('JAX_PLATFORMS') == 'cpu' and 'jax' not in sys.modules:
    del os.environ['JAX_PLATFORMS']

import math
import numpy as np

T_MAX, BATCH, D, C = 512, 128, 256, 7
NCORE = 8
NCONV = BATCH // NCORE          # conversations per core

_BUILD_CACHE = {}


def _f8(x):
    """Host fp32 -> e4m3 bytes (clipped to the TRN-compatible +-240 range)."""
    import ml_dtypes
    return np.ascontiguousarray(
        np.clip(np.asarray(x, np.float32), -240.0, 240.0)
        .astype(ml_dtypes.float8_e4m3fn).view(np.uint8))


def _pow2_scale(w):
    s = float(np.std(w))
    if s == 0.0 or not np.isfinite(s):
        return 1.0
    return float(2.0 ** round(math.log2(4.0 / s)))


def _build(with_bias, slot_lens, scales):
    """Build + compile the SPMD Bass program. Returns the Bacc instance."""
    from contextlib import ExitStack
    import concourse.bacc as bacc
    import concourse.bass as bass  # noqa: F401
    from concourse import mybir, tile

    f32 = mybir.dt.float32
    bf16 = mybir.dt.bfloat16
    fp8 = mybir.dt.float8e4
    u8 = mybir.dt.uint8
    AF = mybir.ActivationFunctionType
    ALU = mybir.AluOpType
    AX = mybir.AxisListType
    PM = mybir.MatmulPerfMode.DoubleRow

    ws_e = {0: scales['ws_e_s'], 1: scales['ws_e_p']}
    ws_h = {0: scales['ws_h_s'], 1: scales['ws_h_p']}
    ows = scales['ows']

    nc = bacc.Bacc("TRN2", target_bir_lowering=False, debug=False,
                   num_devices=NCORE)

    def din(name, shape, dt):
        return nc.dram_tensor(name, shape, dt, kind="ExternalInput").ap()

    xt_d = {0: din("xts", [NCONV, 2, 128, T_MAX], u8),
            1: din("xtp", [NCONV, 2, 128, T_MAX], u8)}
    xn_d = {0: din("xns", [T_MAX, NCONV, D], u8),
            1: din("xnp", [T_MAX, NCONV, D], u8)}
    wdefs = {}
    for sti, st in enumerate(("s", "p")):
        wdefs[sti] = dict(
            we=din(f"we_{st}", [D, 768], u8),
            wh=din(f"wh_{st}", [D, 1024], u8),
            wr=din(f"wr_{st}", [D, 1024], u8),
            b1=din(f"b1_{st}", [128, 6], f32) if with_bias else None,
            b2=din(f"b2_{st}", [128, 8], f32) if with_bias else None,
        )
    m8_d = din("m8", [NCONV, 128, 512], u8)   # 0/1 row-validity, per ut block
    outw_d = din("outw", [4 * D, C], u8)
    out_d = nc.dram_tensor("out", [NCONV, T_MAX, C], f32,
                           kind="ExternalOutput").ap()

    with ExitStack() as ctx:
        tc = ctx.enter_context(tile.TileContext(nc))
        const = ctx.enter_context(tc.tile_pool(name="const", bufs=1))
        xpool = ctx.enter_context(tc.tile_pool(name="xpool", bufs=10))
        work = ctx.enter_context(tc.tile_pool(name="work", bufs=2))
        fpool = ctx.enter_context(tc.tile_pool(name="fpool", bufs=3))
        opool = ctx.enter_context(tc.tile_pool(name="opool", bufs=2))
        gp = ctx.enter_context(tc.tile_pool(name="gp", bufs=2, space="PSUM"))
        ep = ctx.enter_context(tc.tile_pool(name="ep", bufs=2, space="PSUM"))

        # ---- constants / weights --------------------------------------
        W = {}
        for sti, st in enumerate(("s", "p")):
            d = wdefs[sti]
            we_t = const.tile([128, 2, 768], fp8, name=f"we_t{st}")
            nc.sync.dma_start(out=we_t, in_=d["we"].bitcast(fp8).rearrange(
                "(kt p) m -> p kt m", p=128))
            wh_t = const.tile([128, 2, 1024], fp8, name=f"wh_t{st}")
            nc.sync.dma_start(out=wh_t, in_=d["wh"].bitcast(fp8).rearrange(
                "(kt p) m -> p kt m", p=128))
            wr_t = const.tile([128, 2, 1024], fp8, name=f"wr_t{st}")
            nc.sync.dma_start(out=wr_t, in_=d["wr"].bitcast(fp8).rearrange(
                "(kt p) m -> p kt m", p=128))
            b1_t = b2_t = None
            if with_bias:
                b1_t = const.tile([128, 6], f32, name=f"b1_t{st}")
                nc.sync.dma_start(out=b1_t, in_=d["b1"])
                b2_t = const.tile([128, 8], f32, name=f"b2_t{st}")
                nc.sync.dma_start(out=b2_t, in_=d["b2"])
            W[sti] = dict(we=we_t, wh=wh_t, wr=wr_t, b1=b1_t, b2=b2_t)
        m8_t = const.tile([128, NCONV, 4, 128], fp8, name="m8_t")
        nc.sync.dma_start(out=m8_t, in_=m8_d.bitcast(fp8).rearrange(
            "j p (b c) -> p j b c", b=4))
        outw_t = const.tile([128, 8, C], fp8, name="outw_t")
        nc.sync.dma_start(out=outw_t, in_=outw_d.bitcast(fp8).rearrange(
            "(kt p) c -> p kt c", p=128))
        lg_all = const.tile([128, NCONV, 4, C], f32, name="lg_all")
        s_all = const.tile([128, NCONV * 4], f32, name="s_all")
        lnS = const.tile([128, NCONV * 4], f32, name="lnS")

        def mm(ps, lhsT, rhs, start, stop, pm=PM):
            nc.tensor.matmul(ps, lhsT, rhs, start=start, stop=stop,
                             perf_mode=pm)

        def dims(j):
            Lv = int(slot_lens[j])
            # 16-aligned: DoubleRow LDWEIGHTS requires k-pair step % 16 == 0
            L = min(T_MAX, ((Lv + 15) // 16) * 16)
            UT = (Lv + 127) // 128
            return Lv, L, UT

        S = [None] * NCONV      # per-conv pipeline state

        def gate_act(pg_t, out_t, L, scale, bias_t, bcol):
            """[128,2,L] psum -> bf16; merged unless per-z biases needed."""
            if with_bias:
                for z in range(2):
                    nc.scalar.activation(out_t[:, z, :], pg_t[:, z, 0:L],
                                         AF.Tanh, scale=scale,
                                         bias=bias_t[:, bcol + z:bcol + z + 1])
            else:
                nc.scalar.activation(out_t[:, :, :], pg_t[:, :, 0:L],
                                     AF.Tanh, scale=scale)

        def emit_dma(j):
            Lv, L, UT = dims(j)
            LX = UT * 128
            xt_, xn_ = {}, {}
            for st in (0, 1):
                xt = xpool.tile([128, 2, LX], fp8, tag="xt", name=f"xt{j}_{st}")
                for kd in range(2):
                    nc.sync.dma_start(out=xt[:, kd, :],
                                      in_=xt_d[st].bitcast(fp8)[j, kd, :, 0:LX])
                xn = xpool.tile([128, 4, D], fp8, tag="xn", name=f"xn{j}_{st}")
                for ut in range(UT):
                    nc.sync.dma_start(
                        out=xn[:, ut, :],
                        in_=xn_d[st].bitcast(fp8)[ut * 128:(ut + 1) * 128, j, :])
                xt_[st], xn_[st] = xt, xn
            S[j] = dict(xt=xt_, xn=xn_)

        def emit_F(j):
            Lv, L, UT = dims(j)
            st_ = S[j]
            g1_ = {}
            for st in (0, 1):
                w = W[st]
                gts = []
                for gi in range(3):            # i, g, o
                    pg_t = gp.tile([128, 2, T_MAX], f32, tag="pg",
                                   name=f"pg1{j}_{st}_{gi}")
                    for z in range(2):
                        mm(pg_t[:, z, 0:L],
                           w["we"][:, :, (2 * gi + z) * 128:(2 * gi + z + 1) * 128],
                           st_['xt'][st][:, :, 0:L], True, True)
                    gt = work.tile([128, 2, L], bf16, tag="g1", bufs=8,
                                   name=f"g1{j}_{st}_{gi}")
                    gate_act(pg_t, gt, L, 0.5 / ws_e[st], w["b1"], 2 * gi)
                    gts.append(gt)
                g1_[st] = gts
            cs1_, th1_, hs1_ = {}, {}, {}
            for st in (0, 1):
                cs1 = work.tile([128, 2, L], bf16, tag="cs", bufs=8,
                                name=f"cs1{j}_{st}")
                nc.vector.scalar_tensor_tensor(cs1[:, :, :],
                                               g1_[st][0][:, :, :], 1.0,
                                               g1_[st][1][:, :, :],
                                               ALU.add, ALU.mult)
                cs1_[st] = cs1
            for st in (0, 1):
                th1 = work.tile([128, 2, L], bf16, tag="th", bufs=4,
                                name=f"th1{j}_{st}")
                nc.scalar.activation(th1[:, :, :], cs1_[st][:, :, :],
                                     AF.Tanh, scale=0.5)
                th1_[st] = th1
            for st in (0, 1):
                hs1 = work.tile([128, 2, L], fp8, tag="hs", bufs=12,
                                name=f"hs1{j}_{st}")
                nc.vector.scalar_tensor_tensor(hs1[:, :, :],
                                               g1_[st][2][:, :, :], 1.0,
                                               th1_[st][:, :, :],
                                               ALU.add, ALU.mult)
                hs1_[st] = hs1
            st_.update(cs1=cs1_, hs1=hs1_)

        def attention(j, st, hs_tile, step, L, UT):
            """A = exp(0.5*e); Z = 1/(m8 . A) — masking via m8/zeroed-xn."""
            xt = S[j]['xt'][st]
            A = work.tile([128, 4, L], fp8, tag="A", bufs=4,
                          name=f"A{j}_{st}_{step}")
            done = 0
            while done < UT:
                take = 2 if UT - done >= 2 else 1
                et = ep.tile([128, 2, T_MAX], f32, tag="e",
                             name=f"e{j}_{st}_{step}_{done}")
                for q in range(take):
                    ut = done + q
                    mm(et[:, q, 0:L], xt[:, :, ut * 128:(ut + 1) * 128],
                       hs_tile[:, :, :], True, True)
                nc.scalar.activation(A[:, done:done + take, :],
                                     et[:, 0:take, 0:L], AF.Exp, scale=0.5)
                done += take
            NPAIR, ODD = UT // 2, UT % 2
            pt = ep.tile([128, 2, T_MAX], f32, tag="e", name=f"ps{j}_{st}_{step}")
            for k in range(NPAIR):
                mm(pt[:, 0, 0:L], m8_t[:, j, 2 * k:2 * k + 2, :],
                   A[:, 2 * k:2 * k + 2, :], k == 0,
                   k == NPAIR - 1 and not ODD)
            if ODD:
                mm(pt[:, 0, 0:L], m8_t[:, j, UT - 1, :], A[:, UT - 1, :],
                   NPAIR == 0, True, pm=None)
            Z = work.tile([128, L], f32, tag="Z", bufs=4, name=f"Z{j}_{st}_{step}")
            nc.vector.reciprocal_approx_fast(Z[:, :], pt[:, 0, 0:L])
            return A, Z

        def r_apply(j, st, A, Z, out_tile, zoff, relu, step, L, UT):
            """out[:, zoff+dt, :] = (X^T A) * Z, optionally relu'd."""
            xn = S[j]['xn'][st]
            NPAIR, ODD = UT // 2, UT % 2
            for dt in range(2):
                rt = ep.tile([128, 2, T_MAX], f32, tag="e",
                             name=f"r{j}_{st}_{step}_{dt}")
                for k in range(NPAIR):
                    mm(rt[:, 0, 0:L],
                       xn[:, 2 * k:2 * k + 2, dt * 128:(dt + 1) * 128],
                       A[:, 2 * k:2 * k + 2, :], k == 0,
                       k == NPAIR - 1 and not ODD)
                if ODD:
                    mm(rt[:, 0, 0:L], xn[:, UT - 1, dt * 128:(dt + 1) * 128],
                       A[:, UT - 1, :], NPAIR == 0, True, pm=None)
                nc.vector.scalar_tensor_tensor(
                    out_tile[:, zoff + dt, :], rt[:, 0, 0:L],
                    0.0 if relu else 1.0, Z[:, :],
                    ALU.max if relu else ALU.mult, ALU.mult)

        def emit_B1(j):
            Lv, L, UT = dims(j)
            st_ = S[j]
            AZ1, r1_ = {}, {}
            for st in (0, 1):
                AZ1[st] = attention(j, st, st_['hs1'][st], 1, L, UT)
            for st in (0, 1):
                r1 = work.tile([128, 2, L], fp8, tag="r1", bufs=8,
                               name=f"r1{j}_{st}")
                r_apply(j, st, AZ1[st][0], AZ1[st][1], r1, 0, False, 1, L, UT)
                r1_[st] = r1
            st_.update(r1=r1_)

        def emit_B2(j):
            Lv, L, UT = dims(j)
            st_ = S[j]
            g2_ = {}
            for st in (0, 1):
                w = W[st]
                gts = []
                for gi in range(4):            # i, f, g, o
                    pg_t = gp.tile([128, 2, T_MAX], f32, tag="pg",
                                   name=f"pg2{j}_{st}_{gi}")
                    for z in range(2):
                        m = 2 * gi + z
                        mm(pg_t[:, z, 0:L],
                           w["wh"][:, :, m * 128:(m + 1) * 128],
                           st_['hs1'][st][:, :, :], True, False)
                        mm(pg_t[:, z, 0:L],
                           w["wr"][:, :, m * 128:(m + 1) * 128],
                           st_['r1'][st][:, :, :], False, True)
                    gt = work.tile([128, 2, L], bf16, tag="g2", bufs=8,
                                   name=f"g2{j}_{st}_{gi}")
                    gate_act(pg_t, gt, L, 0.5 / ws_h[st], w["b2"], 2 * gi)
                    gts.append(gt)
                g2_[st] = gts
            cs2_, th2_, hs2_ = {}, {}, {}
            for st in (0, 1):
                gi2, gf2, gg2, go2 = g2_[st]
                t1 = work.tile([128, 2, L], bf16, tag="tmp", bufs=4,
                               name=f"t1{j}_{st}")
                nc.vector.scalar_tensor_tensor(t1[:, :, :], gf2[:, :, :], 1.0,
                                               st_['cs1'][st][:, :, :],
                                               ALU.add, ALU.mult)
                t2 = work.tile([128, 2, L], bf16, tag="tmp", bufs=4,
                               name=f"t2{j}_{st}")
                nc.vector.scalar_tensor_tensor(t2[:, :, :], gi2[:, :, :], 1.0,
                                               gg2[:, :, :], ALU.add, ALU.mult)
                cs2 = work.tile([128, 2, L], bf16, tag="cs", bufs=8,
                                name=f"cs2{j}_{st}")
                nc.vector.scalar_tensor_tensor(cs2[:, :, :], t1[:, :, :], 0.5,
                                               t2[:, :, :], ALU.mult, ALU.add)
                cs2_[st] = cs2
            for st in (0, 1):
                th2 = work.tile([128, 2, L], bf16, tag="th", bufs=4,
                                name=f"th2{j}_{st}")
                nc.scalar.activation(th2[:, :, :], cs2_[st][:, :, :],
                                     AF.Tanh, scale=0.5)
                th2_[st] = th2
            for st in (0, 1):
                hs2 = work.tile([128, 2, L], fp8, tag="hs", bufs=12,
                                name=f"hs2{j}_{st}")
                nc.vector.scalar_tensor_tensor(hs2[:, :, :],
                                               g2_[st][3][:, :, :], 1.0,
                                               th2_[st][:, :, :],
                                               ALU.add, ALU.mult)
                hs2_[st] = hs2
            st_.update(hs2=hs2_)

        def emit_B3(j):
            Lv, L, UT = dims(j)
            st_ = S[j]
            ft_ = {}
            for st in (0, 1):
                A2, Z2 = attention(j, st, st_['hs2'][st], 2, L, UT)
                ft = fpool.tile([128, 4, L], fp8, tag=f"ft{st}", name=f"ft{j}_{st}")
                nc.vector.tensor_scalar_max(ft[:, 0:2, :],
                                            st_['hs2'][st][:, :, :], 0.0)
                r_apply(j, st, A2, Z2, ft, 2, True, 2, L, UT)
                ft_[st] = ft
            # logits transposed: [t, C] per 128-t chunk (8-wide slots in psum)
            lpt = ep.tile([128, 2, T_MAX], f32, tag="e", name=f"lp{j}")
            for cch in range(UT):
                ncch = min(128, L - cch * 128)
                for m, (ftile, zz) in enumerate(
                        ((ft_[0], 0), (ft_[0], 2), (ft_[1], 0), (ft_[1], 2))):
                    mm(lpt[0:ncch, 0, cch * 8:cch * 8 + C],
                       ftile[:, zz:zz + 2, cch * 128:cch * 128 + ncch],
                       outw_t[:, 2 * m:2 * m + 2, :], m == 0, m == 3)
            lgv = lpt[:, 0, 0:UT * 8].rearrange("p (u c) -> p u c", c=8)[:, :, 0:C]
            nc.vector.tensor_scalar_mul(lg_all[:, j, 0:UT, :], lgv, 1.0 / ows)
            elg = work.tile([128, 4, C], f32, tag="elg", name=f"elg{j}")
            nc.scalar.activation(elg[:, 0:UT, :], lgv, AF.Exp, scale=1.0 / ows)
            nc.vector.tensor_reduce(s_all[:, j * 4:j * 4 + UT],
                                    elg[:, 0:UT, :], AX.X, ALU.add)
            S[j] = None

        # ---- 4-stage software-pipelined main loop ----------------------
        emit_dma(0)
        for t in range(NCONV + 3):
            if t + 1 < NCONV:
                emit_dma(t + 1)
            if t >= 3:
                emit_B3(t - 3)
            if t >= 2 and t - 2 < NCONV:
                emit_B2(t - 2)
            if t >= 1 and t - 1 < NCONV:
                emit_B1(t - 1)
            if t < NCONV:
                emit_F(t)

        # ---- final: logp = lg - ln(rowsum) ----------------------------
        nc.scalar.activation(lnS[:, :], s_all[:, :], AF.Ln)
        for j in range(NCONV):
            UT = dims(j)[2]
            ot = opool.tile([128, 4, C], f32, tag="ot", name=f"ot{j}")
            for cch in range(UT):
                nc.vector.tensor_scalar_sub(ot[:, cch, :], lg_all[:, j, cch, :],
                                            lnS[:, j * 4 + cch:j * 4 + cch + 1])
            nc.sync.dma_start(
                out=out_d[j].rearrange("(c p) k -> p c k", p=128)[:, 0:UT, :],
                in_=ot[:, 0:UT, :])

    nc.compile()
    return nc


def _host_prep(inputs):
    """Fold weights, quantize to fp8, pick conversation->core assignment."""
    x_s = np.asarray(inputs["input"], dtype=np.float32)
    x_p = np.asarray(inputs["speakers"], dtype=np.float32)
    lengths = np.asarray(inputs["utterance_lengths"]).astype(np.int64)
    fc_w = np.asarray(inputs["fc_w"], dtype=np.float32)
    fc_b = np.asarray(inputs["fc_b"], dtype=np.float32)
    out_w = np.asarray(inputs["out_w"], dtype=np.float32)
    out_b = np.asarray(inputs["out_b"], dtype=np.float32)

    per_stream = {}
    scales = {}
    any_b = False
    for st in ("s", "p"):
        w_ih = np.asarray(inputs[f"w_ih_{st}"], dtype=np.float32)
        w_hh = np.asarray(inputs[f"w_hh_{st}"], dtype=np.float32)
        b_ih = np.asarray(inputs[f"b_ih_{st}"], dtype=np.float32)
        b_hh = np.asarray(inputs[f"b_hh_{st}"], dtype=np.float32)
        W_eff = w_ih @ fc_w                          # [1024, 256]
        bias1 = w_ih @ fc_b + b_ih + b_hh            # [1024]
        sel = np.r_[0:D, 2 * D:4 * D]                # i, g, o rows
        We = np.ascontiguousarray(W_eff[sel].T)      # [256, 768]
        We[:, D:2 * D] *= 2.0                        # g-gate doubling
        Wh = np.ascontiguousarray((0.5 * (w_ih[:, :D] + w_hh)).T)  # [256, 1024]
        Wr = np.ascontiguousarray(w_ih[:, D:].T)     # [256, 1024]
        Wh[:, 2 * D:3 * D] *= 2.0
        Wr[:, 2 * D:3 * D] *= 2.0
        ws_e = _pow2_scale(We)
        ws_h = _pow2_scale(np.concatenate([Wh, Wr], axis=0))
        scales[f'ws_e_{st}'] = ws_e
        scales[f'ws_h_{st}'] = ws_h
        # per-slice activation biases (pre-multiplied by the tanh input
        # scale: 0.5 normally, 1.0 for the doubled g-gate)
        b1_sel = bias1[sel]                          # [768] i,g,o
        bias2 = b_ih + b_hh                          # [1024] i,f,g,o
        b1_cols = np.zeros((128, 6), np.float32)
        for m in range(6):
            f = 1.0 if m in (2, 3) else 0.5
            b1_cols[:, m] = f * b1_sel[m * 128:(m + 1) * 128]
        b2_cols = np.zeros((128, 8), np.float32)
        for m in range(8):
            f = 1.0 if m in (4, 5) else 0.5
            b2_cols[:, m] = f * bias2[m * 128:(m + 1) * 128]
        any_b |= bool(np.any(b1_cols != 0.0) or np.any(b2_cols != 0.0))
        per_stream[st] = (_f8(We * ws_e), _f8(Wh * ws_h), _f8(Wr * ws_h),
                          b1_cols, b2_cols)

    # out_w columns for the h-halves get the 0.5 compensation (h stored as 2h)
    ow = out_w.copy()
    ow[:, 0:D] *= 0.5
    ow[:, 2 * D:3 * D] *= 0.5
    ows = _pow2_scale(ow)
    scales['ows'] = ows
    outw8 = _f8(ow.T * ows)                          # [1024, 7]
    host_out_b = out_b

    # conversation -> (core, slot): sort by length desc, round-robin
    order = np.argsort(-lengths, kind="stable")
    assign = {}
    for rank, conv in enumerate(order):
        assign[int(conv)] = (rank % NCORE, rank // NCORE)
    order_lens = lengths[order]
    slot_lens = tuple(int(order_lens[8 * k]) for k in range(NCONV))

    # fp8-quantize the banks once (identical bytes for both layouts)
    import ml_dtypes
    xs8 = np.clip(x_s, -240.0, 240.0).astype(ml_dtypes.float8_e4m3fn).view(np.uint8)
    xp8 = np.clip(x_p, -240.0, 240.0).astype(ml_dtypes.float8_e4m3fn).view(np.uint8)
    one8 = int(np.array([1.0], dtype=ml_dtypes.float8_e4m3fn).view(np.uint8)[0])

    in_maps = []
    core_convs = []
    for core in range(NCORE):
        ids = [None] * NCONV
        for conv, (c, s) in assign.items():
            if c == core:
                ids[s] = conv
        core_convs.append(ids)
        m8 = np.zeros((NCONV, 128, 512), dtype=np.uint8)
        xns = xs8[:, ids, :].copy()      # [T_MAX, NCONV, D], u-major
        xnp = xp8[:, ids, :].copy()
        for s, conv in enumerate(ids):
            Lc = int(lengths[conv])
            valid = (np.arange(T_MAX) < Lc)
            m8[s, :, :] = np.where(valid, one8, 0).astype(np.uint8).reshape(
                4, 128).T.repeat(128, axis=1).reshape(128, 512)
            xns[Lc:, s, :] = 0
            xnp[Lc:, s, :] = 0
        im = {
            "xts": np.ascontiguousarray(
                xs8[:, ids, :].transpose(1, 2, 0).reshape(NCONV, 2, 128, T_MAX)),
            "xtp": np.ascontiguousarray(
                xp8[:, ids, :].transpose(1, 2, 0).reshape(NCONV, 2, 128, T_MAX)),
            "xns": np.ascontiguousarray(xns),
            "xnp": np.ascontiguousarray(xnp),
            "m8": m8,
            "outw": outw8,
        }
        for st in ("s", "p"):
            We8, Wh8, Wr8, b1c, b2c = per_stream[st]
            im[f"we_{st}"] = We8
            im[f"wh_{st}"] = Wh8
            im[f"wr_{st}"] = Wr8
            if any_b:
                im[f"b1_{st}"] = b1c
                im[f"b2_{st}"] = b2c
        in_maps.append(im)
    key = (any_b, slot_lens,
           tuple(sorted((k, float(v)) for k, v in scales.items())))
    return in_maps, core_convs, lengths, key, scales, host_out_b


def _gather(results, core_convs, lengths, out_b):
    """results: per-core {'out': [NCONV, T_MAX, C]} -> [sum(len), C]."""
    where = {}
    for core, ids in enumerate(core_convs):
        for slot, conv in enumerate(ids):
            where[conv] = (core, slot)
    chunks = []
    nz = bool(np.any(out_b != 0.0))
    for b in range(BATCH):
        core, slot = where[b]
        L = int(lengths[b])
        lg = results[core]["out"][slot, :L, :]
        if nz:
            # device log-softmax omitted out_b; log_softmax is shift-invariant
            # per row, so redo it with the bias added.
            lg = lg + out_b[None, :]
            lg = lg - np.log(np.exp(lg).sum(axis=1, keepdims=True))
        chunks.append(np.ascontiguousarray(lg))
    return np.concatenate(chunks, axis=0).astype(np.float32)


def _get_nc(key, scales):
    if key not in _BUILD_CACHE:
        _BUILD_CACHE[key] = _build(key[0], key[1], scales)
    return _BUILD_CACHE[key]


def kernel(**inputs):
    from concourse import bass_utils
    in_maps, core_convs, lengths, key, scales, out_b = _host_prep(inputs)
    nc = _get_nc(key, scales)
    res = bass_utils.run_bass_kernel_spmd(nc, in_maps, core_ids=list(range(NCORE)))
    return _gather(res.results, core_convs, lengths, out_b)


# revision 20
# speedup vs baseline: 1.7456x; 1.0307x over previous
"""DCRNCognition Trainium2 kernel — fp8 DoubleRow edition.

Self-contained: builds a Bass/Tile SPMD program for 8 NeuronCores, shards the
batch (conversation) axis across cores, runs via run_bass_kernel_spmd, and
gathers the valid positions on the host.

Math restructuring (identical to the verified baseline, rel err ~9e-7 in f32):
  - fc layer folded into step-1 LSTM gates; step-1 f-gate/c-init dead
  - step-2: gates2 = hs1 @ Wh.T + r1 @ Wr.T  (Wh = 0.5*(w_ih[:, :D]+w_hh))
  - softmax normalization deferred to r:  r = (X^T A) * (1/sum_u A)
  - sigmoid via tanh; h,c carried scaled by 2 (hs=2h, cs=2c)

Precision plan (validated on host: rel err ~6e-4 vs the 2e-2 gate):
  - ALL matmuls fp8 e4m3 with DoubleRow perf mode, fp32 PSUM accum.
  - weights pre-scaled by a power of 2 into fp8 range on the host; the
    compensation folds into the free activation `scale` operands.
  - g-gate weight rows doubled on host so every gate activation shares
    scale=0.5 -> one ACT instruction per gate pair.
  - masking without exp bias: invalid bank rows (u >= len) are zeroed on
    the host in the u-major layout (kills their r contribution) and the
    softmax denominator contracts A against a per-conversation 0/1 mask
    as the matmul stationary (kills them in the sum).
  - logits computed transposed ([t,C]): log-softmax along the free axis,
    one batched Ln at the end -> 2 ACT table loads total.

Scheduling: 4-stage software pipeline (F=gates1+cell1, B1=attention1,
B2=gates2+cell2, B3=attention2+logits) with a 3-conversation skew so every
engine queue always holds ready work. PSUM: two 4-bank pools (gates /
attention+logits).
"""
import os
import sys
sys.path.insert(0, '/opt/trn_rl_repo')

# run_bass_kernel_spmd executes through jax/PJRT on the axon-tunneled
# NeuronCores; a JAX_PLATFORMS=cpu pin would hide them.
if os.environ.get('JAX_PLATFORMS') == 'cpu' and 'jax' not in sys.modules:
    del os.environ['JAX_PLATFORMS']

import math
import numpy as np

T_MAX, BATCH, D, C = 512, 128, 256, 7
NCORE = 8
NCONV = BATCH // NCORE          # conversations per core

_BUILD_CACHE = {}


def _f8(x):
    """Host fp32 -> e4m3 bytes (clipped to the TRN-compatible +-240 range)."""
    import ml_dtypes
    return np.ascontiguousarray(
        np.clip(np.asarray(x, np.float32), -240.0, 240.0)
        .astype(ml_dtypes.float8_e4m3fn).view(np.uint8))


def _pow2_scale(w):
    s = float(np.std(w))
    if s == 0.0 or not np.isfinite(s):
        return 1.0
    return float(2.0 ** round(math.log2(4.0 / s)))


def _build(with_bias, slot_lens, scales):
    """Build + compile the SPMD Bass program. Returns the Bacc instance."""
    from contextlib import ExitStack
    import concourse.bacc as bacc
    import concourse.bass as bass  # noqa: F401
    from concourse import mybir, tile

    f32 = mybir.dt.float32
    bf16 = mybir.dt.bfloat16
    fp8 = mybir.dt.float8e4
    u8 = mybir.dt.uint8
    AF = mybir.ActivationFunctionType
    ALU = mybir.AluOpType
    AX = mybir.AxisListType
    PM = mybir.MatmulPerfMode.DoubleRow

    ws_e = {0: scales['ws_e_s'], 1: scales['ws_e_p']}
    ws_h = {0: scales['ws_h_s'], 1: scales['ws_h_p']}
    ows = scales['ows']

    nc = bacc.Bacc("TRN2", target_bir_lowering=False, debug=False,
                   num_devices=NCORE)

    def din(name, shape, dt):
        return nc.dram_tensor(name, shape, dt, kind="ExternalInput").ap()

    xt_d = {0: din("xts", [NCONV, 2, 128, T_MAX], u8),
            1: din("xtp", [NCONV, 2, 128, T_MAX], u8)}
    xn_d = {0: din("xns", [T_MAX, NCONV, D], u8),
            1: din("xnp", [T_MAX, NCONV, D], u8)}
    wdefs = {}
    for sti, st in enumerate(("s", "p")):
        wdefs[sti] = dict(
            we=din(f"we_{st}", [D, 768], u8),
            wh=din(f"wh_{st}", [D, 1024], u8),
            wr=din(f"wr_{st}", [D, 1024], u8),
            b1=din(f"b1_{st}", [128, 6], f32) if with_bias else None,
            b2=din(f"b2_{st}", [128, 8], f32) if with_bias else None,
        )
    m8_d = din("m8", [NCONV, 128, 512], u8)   # 0/1 row-validity, per ut block
    outw_d = din("outw", [4 * D, C], u8)
    out_d = nc.dram_tensor("out", [NCONV, T_MAX, C], f32,
                           kind="ExternalOutput").ap()

    with ExitStack() as ctx:
        tc = ctx.enter_context(tile.TileContext(nc))
        const = ctx.enter_context(tc.tile_pool(name="const", bufs=1))
        xpool = ctx.enter_context(tc.tile_pool(name="xpool", bufs=10))
        work = ctx.enter_context(tc.tile_pool(name="work", bufs=2))
        fpool = ctx.enter_context(tc.tile_pool(name="fpool", bufs=3))
        opool = ctx.enter_context(tc.tile_pool(name="opool", bufs=2))
        gp = ctx.enter_context(tc.tile_pool(name="gp", bufs=1, space="PSUM"))
        ep = ctx.enter_context(tc.tile_pool(name="ep", bufs=2, space="PSUM"))

        # ---- constants / weights --------------------------------------
        W = {}
        for sti, st in enumerate(("s", "p")):
            d = wdefs[sti]
            we_t = const.tile([128, 2, 768], fp8, name=f"we_t{st}")
            nc.sync.dma_start(out=we_t, in_=d["we"].bitcast(fp8).rearrange(
                "(kt p) m -> p kt m", p=128))
            wh_t = const.tile([128, 2, 1024], fp8, name=f"wh_t{st}")
            nc.sync.dma_start(out=wh_t, in_=d["wh"].bitcast(fp8).rearrange(
                "(kt p) m -> p kt m", p=128))
            wr_t = const.tile([128, 2, 1024], fp8, name=f"wr_t{st}")
            nc.sync.dma_start(out=wr_t, in_=d["wr"].bitcast(fp8).rearrange(
                "(kt p) m -> p kt m", p=128))
            b1_t = b2_t = None
            if with_bias:
                b1_t = const.tile([128, 6], f32, name=f"b1_t{st}")
                nc.sync.dma_start(out=b1_t, in_=d["b1"])
                b2_t = const.tile([128, 8], f32, name=f"b2_t{st}")
                nc.sync.dma_start(out=b2_t, in_=d["b2"])
            W[sti] = dict(we=we_t, wh=wh_t, wr=wr_t, b1=b1_t, b2=b2_t)
        m8_t = const.tile([128, NCONV, 4, 128], fp8, name="m8_t")
        nc.sync.dma_start(out=m8_t, in_=m8_d.bitcast(fp8).rearrange(
            "j p (b c) -> p j b c", b=4))
        outw_t = const.tile([128, 8, C], fp8, name="outw_t")
        nc.sync.dma_start(out=outw_t, in_=outw_d.bitcast(fp8).rearrange(
            "(kt p) c -> p kt c", p=128))
        lg_all = const.tile([128, NCONV, 4, C], f32, name="lg_all")
        s_all = const.tile([128, NCONV * 4], f32, name="s_all")
        lnS = const.tile([128, NCONV * 4], f32, name="lnS")

        def mm(ps, lhsT, rhs, start, stop, pm=PM):
            nc.tensor.matmul(ps, lhsT, rhs, start=start, stop=stop,
                             perf_mode=pm)

        def dims(j):
            Lv = int(slot_lens[j])
            # 16-aligned: DoubleRow LDWEIGHTS requires k-pair step % 16 == 0
            L = min(T_MAX, ((Lv + 15) // 16) * 16)
            UT = (Lv + 127) // 128
            return Lv, L, UT

        S = [None] * NCONV      # per-conv pipeline state

        def gate_act(pg_ap, out_ap, nsl, scale, bias_t, bcol):
            """[128,nsl,L] psum -> bf16; merged unless per-z biases needed."""
            if with_bias:
                for z in range(nsl):
                    nc.scalar.activation(out_ap[:, z, :], pg_ap[:, z, :],
                                         AF.Tanh, scale=scale,
                                         bias=bias_t[:, bcol + z:bcol + z + 1])
            else:
                nc.scalar.activation(out_ap, pg_ap, AF.Tanh, scale=scale)

        def emit_dma(j):
            Lv, L, UT = dims(j)
            LX = UT * 128
            xt_, xn_ = {}, {}
            for st in (0, 1):
                xt = xpool.tile([128, 2, LX], fp8, tag="xt", name=f"xt{j}_{st}")
                for kd in range(2):
                    nc.sync.dma_start(out=xt[:, kd, :],
                                      in_=xt_d[st].bitcast(fp8)[j, kd, :, 0:LX])
                xn = xpool.tile([128, 4, D], fp8, tag="xn", name=f"xn{j}_{st}")
                for ut in range(UT):
                    nc.sync.dma_start(
                        out=xn[:, ut, :],
                        in_=xn_d[st].bitcast(fp8)[ut * 128:(ut + 1) * 128, j, :])
                xt_[st], xn_[st] = xt, xn
            S[j] = dict(xt=xt_, xn=xn_)

        def emit_F(j):
            Lv, L, UT = dims(j)
            st_ = S[j]
            g1_ = {}
            for st in (0, 1):
                w = W[st]
                pgA = gp.tile([128, 4, T_MAX], f32, tag="pg", name=f"pg1a{j}_{st}")
                for m in range(4):             # i0 i1 g0 g1
                    mm(pgA[:, m, 0:L], w["we"][:, :, m * 128:(m + 1) * 128],
                       st_['xt'][st][:, :, 0:L], True, True)
                gig = work.tile([128, 4, L], bf16, tag="g1", bufs=6,
                                name=f"g1ig{j}_{st}")
                gate_act(pgA[:, :, 0:L], gig[:, :, :], 4, 0.5 / ws_e[st],
                         w["b1"], 0)
                pgB = gp.tile([128, 4, T_MAX], f32, tag="pg", name=f"pg1b{j}_{st}")
                for m in (4, 5):               # o0 o1
                    mm(pgB[:, m - 4, 0:L], w["we"][:, :, m * 128:(m + 1) * 128],
                       st_['xt'][st][:, :, 0:L], True, True)
                go = work.tile([128, 2, L], bf16, tag="go", bufs=6,
                               name=f"g1o{j}_{st}")
                gate_act(pgB[:, 0:2, 0:L], go[:, :, :], 2, 0.5 / ws_e[st],
                         w["b1"], 4)
                g1_[st] = (gig, go)
            cs1 = work.tile([128, 4, L], bf16, tag="cs", bufs=6, name=f"cs1{j}")
            for st in (0, 1):
                nc.vector.scalar_tensor_tensor(cs1[:, 2 * st:2 * st + 2, :],
                                               g1_[st][0][:, 0:2, :], 1.0,
                                               g1_[st][0][:, 2:4, :],
                                               ALU.add, ALU.mult)
            th1 = work.tile([128, 4, L], bf16, tag="th", bufs=3, name=f"th1{j}")
            nc.scalar.activation(th1[:, :, :], cs1[:, :, :], AF.Tanh, scale=0.5)
            hs1_ = {}
            for st in (0, 1):
                hs1 = work.tile([128, 2, L], fp8, tag="hs", bufs=12,
                                name=f"hs1{j}_{st}")
                nc.vector.scalar_tensor_tensor(hs1[:, :, :],
                                               g1_[st][1][:, :, :], 1.0,
                                               th1[:, 2 * st:2 * st + 2, :],
                                               ALU.add, ALU.mult)
                hs1_[st] = hs1
            st_.update(cs1=cs1, hs1=hs1_)

        def attention(j, st, hs_tile, step, L, UT):
            """A = exp(0.5*e); Z = 1/(m8 . A) — masking via m8/zeroed-xn."""
            xt = S[j]['xt'][st]
            A = work.tile([128, 4, L], fp8, tag="A", bufs=4,
                          name=f"A{j}_{st}_{step}")
            done = 0
            while done < UT:
                take = 2 if UT - done >= 2 else 1
                et = ep.tile([128, 2, T_MAX], f32, tag="e",
                             name=f"e{j}_{st}_{step}_{done}")
                for q in range(take):
                    ut = done + q
                    mm(et[:, q, 0:L], xt[:, :, ut * 128:(ut + 1) * 128],
                       hs_tile[:, :, :], True, True)
                nc.scalar.activation(A[:, done:done + take, :],
                                     et[:, 0:take, 0:L], AF.Exp, scale=0.5)
                done += take
            NPAIR, ODD = UT // 2, UT % 2
            pt = ep.tile([128, 2, T_MAX], f32, tag="e", name=f"ps{j}_{st}_{step}")
            for k in range(NPAIR):
                mm(pt[:, 0, 0:L], m8_t[:, j, 2 * k:2 * k + 2, :],
                   A[:, 2 * k:2 * k + 2, :], k == 0,
                   k == NPAIR - 1 and not ODD)
            if ODD:
                mm(pt[:, 0, 0:L], m8_t[:, j, UT - 1, :], A[:, UT - 1, :],
                   NPAIR == 0, True, pm=None)
            Z = work.tile([128, L], f32, tag="Z", bufs=4, name=f"Z{j}_{st}_{step}")
            nc.vector.reciprocal_approx_fast(Z[:, :], pt[:, 0, 0:L])
            return A, Z

        def r_apply(j, st, A, Z, out_tile, zoff, relu, step, L, UT):
            """out[:, zoff+dt, :] = (X^T A) * Z, optionally relu'd."""
            xn = S[j]['xn'][st]
            NPAIR, ODD = UT // 2, UT % 2
            for dt in range(2):
                rt = ep.tile([128, 2, T_MAX], f32, tag="e",
                             name=f"r{j}_{st}_{step}_{dt}")
                for k in range(NPAIR):
                    mm(rt[:, 0, 0:L],
                       xn[:, 2 * k:2 * k + 2, dt * 128:(dt + 1) * 128],
                       A[:, 2 * k:2 * k + 2, :], k == 0,
                       k == NPAIR - 1 and not ODD)
                if ODD:
                    mm(rt[:, 0, 0:L], xn[:, UT - 1, dt * 128:(dt + 1) * 128],
                       A[:, UT - 1, :], NPAIR == 0, True, pm=None)
                nc.vector.scalar_tensor_tensor(
                    out_tile[:, zoff + dt, :], rt[:, 0, 0:L],
                    0.0 if relu else 1.0, Z[:, :],
                    ALU.max if relu else ALU.mult, ALU.mult)

        def emit_B1(j):
            Lv, L, UT = dims(j)
            st_ = S[j]
            AZ1, r1_ = {}, {}
            for st in (0, 1):
                AZ1[st] = attention(j, st, st_['hs1'][st], 1, L, UT)
            for st in (0, 1):
                r1 = work.tile([128, 2, L], fp8, tag="r1", bufs=8,
                               name=f"r1{j}_{st}")
                r_apply(j, st, AZ1[st][0], AZ1[st][1], r1, 0, False, 1, L, UT)
                r1_[st] = r1
            st_.update(r1=r1_)

        def emit_B2(j):
            Lv, L, UT = dims(j)
            st_ = S[j]
            g2_ = {}
            for st in (0, 1):
                w = W[st]
                gts = []
                for half in range(2):          # (i,f) then (g,o)
                    pg_t = gp.tile([128, 4, T_MAX], f32, tag="pg",
                                   name=f"pg2{j}_{st}_{half}")
                    for z in range(4):
                        m = 4 * half + z
                        mm(pg_t[:, z, 0:L],
                           w["wh"][:, :, m * 128:(m + 1) * 128],
                           st_['hs1'][st][:, :, :], True, False)
                        mm(pg_t[:, z, 0:L],
                           w["wr"][:, :, m * 128:(m + 1) * 128],
                           st_['r1'][st][:, :, :], False, True)
                    gt = work.tile([128, 4, L], bf16, tag="g2", bufs=6,
                                   name=f"g2{j}_{st}_{half}")
                    gate_act(pg_t[:, :, 0:L], gt[:, :, :], 4, 0.5 / ws_h[st],
                             w["b2"], 4 * half)
                    gts.append(gt)
                g2_[st] = gts                  # [if_tile, go_tile]
            cs2 = work.tile([128, 4, L], bf16, tag="cs", bufs=6, name=f"cs2{j}")
            for st in (0, 1):
                gif, ggo = g2_[st]
                t1 = work.tile([128, 2, L], bf16, tag="tmp", bufs=4,
                               name=f"t1{j}_{st}")
                nc.vector.scalar_tensor_tensor(t1[:, :, :], gif[:, 2:4, :], 1.0,
                                               st_['cs1'][:, 2 * st:2 * st + 2, :],
                                               ALU.add, ALU.mult)
                t2 = work.tile([128, 2, L], bf16, tag="tmp", bufs=4,
                               name=f"t2{j}_{st}")
                nc.vector.scalar_tensor_tensor(t2[:, :, :], gif[:, 0:2, :], 1.0,
                                               ggo[:, 0:2, :], ALU.add, ALU.mult)
                nc.vector.scalar_tensor_tensor(cs2[:, 2 * st:2 * st + 2, :],
                                               t1[:, :, :], 0.5, t2[:, :, :],
                                               ALU.mult, ALU.add)
            th2 = work.tile([128, 4, L], bf16, tag="th", bufs=3, name=f"th2{j}")
            nc.scalar.activation(th2[:, :, :], cs2[:, :, :], AF.Tanh, scale=0.5)
            hs2_ = {}
            for st in (0, 1):
                hs2 = work.tile([128, 2, L], fp8, tag="hs", bufs=12,
                                name=f"hs2{j}_{st}")
                nc.vector.scalar_tensor_tensor(hs2[:, :, :],
                                               g2_[st][1][:, 2:4, :], 1.0,
                                               th2[:, 2 * st:2 * st + 2, :],
                                               ALU.add, ALU.mult)
                hs2_[st] = hs2
            st_.update(hs2=hs2_)

        def emit_B3(j):
            Lv, L, UT = dims(j)
            st_ = S[j]
            ft_ = {}
            for st in (0, 1):
                A2, Z2 = attention(j, st, st_['hs2'][st], 2, L, UT)
                ft = fpool.tile([128, 4, L], fp8, tag=f"ft{st}", name=f"ft{j}_{st}")
                nc.vector.tensor_scalar_max(ft[:, 0:2, :],
                                            st_['hs2'][st][:, :, :], 0.0)
                r_apply(j, st, A2, Z2, ft, 2, True, 2, L, UT)
                ft_[st] = ft
            # logits transposed: [t, C] per 128-t chunk (8-wide slots in psum)
            lpt = ep.tile([128, 2, T_MAX], f32, tag="e", name=f"lp{j}")
            for cch in range(UT):
                ncch = min(128, L - cch * 128)
                for m, (ftile, zz) in enumerate(
                        ((ft_[0], 0), (ft_[0], 2), (ft_[1], 0), (ft_[1], 2))):
                    mm(lpt[0:ncch, 0, cch * 8:cch * 8 + C],
                       ftile[:, zz:zz + 2, cch * 128:cch * 128 + ncch],
                       outw_t[:, 2 * m:2 * m + 2, :], m == 0, m == 3)
            lgv = lpt[:, 0, 0:UT * 8].rearrange("p (u c) -> p u c", c=8)[:, :, 0:C]
            nc.vector.tensor_scalar_mul(lg_all[:, j, 0:UT, :], lgv, 1.0 / ows)
            elg = work.tile([128, 4, C], f32, tag="elg", name=f"elg{j}")
            nc.scalar.activation(elg[:, 0:UT, :], lgv, AF.Exp, scale=1.0 / ows)
            nc.vector.tensor_reduce(s_all[:, j * 4:j * 4 + UT],
                                    elg[:, 0:UT, :], AX.X, ALU.add)
            S[j] = None

        # ---- 4-stage software-pipelined main loop ----------------------
        emit_dma(0)
        for t in range(NCONV + 3):
            if t + 1 < NCONV:
                emit_dma(t + 1)
            if t >= 3:
                emit_B3(t - 3)
            if t >= 2 and t - 2 < NCONV:
                emit_B2(t - 2)
            if t >= 1 and t - 1 < NCONV:
                emit_B1(t - 1)
            if t < NCONV:
                emit_F(t)

        # ---- final: logp = lg - ln(rowsum) ----------------------------
        nc.scalar.activation(lnS[:, :], s_all[:, :], AF.Ln)
        for j in range(NCONV):
            UT = dims(j)[2]
            ot = opool.tile([128, 4, C], f32, tag="ot", name=f"ot{j}")
            for cch in range(UT):
                nc.gpsimd.tensor_scalar_sub(ot[:, cch, :], lg_all[:, j, cch, :],
                                            lnS[:, j * 4 + cch:j * 4 + cch + 1])
            nc.sync.dma_start(
                out=out_d[j].rearrange("(c p) k -> p c k", p=128)[:, 0:UT, :],
                in_=ot[:, 0:UT, :])

    nc.compile()
    return nc


def _host_prep(inputs):
    """Fold weights, quantize to fp8, pick conversation->core assignment."""
    x_s = np.asarray(inputs["input"], dtype=np.float32)
    x_p = np.asarray(inputs["speakers"], dtype=np.float32)
    lengths = np.asarray(inputs["utterance_lengths"]).astype(np.int64)
    fc_w = np.asarray(inputs["fc_w"], dtype=np.float32)
    fc_b = np.asarray(inputs["fc_b"], dtype=np.float32)
    out_w = np.asarray(inputs["out_w"], dtype=np.float32)
    out_b = np.asarray(inputs["out_b"], dtype=np.float32)

    per_stream = {}
    scales = {}
    any_b = False
    for st in ("s", "p"):
        w_ih = np.asarray(inputs[f"w_ih_{st}"], dtype=np.float32)
        w_hh = np.asarray(inputs[f"w_hh_{st}"], dtype=np.float32)
        b_ih = np.asarray(inputs[f"b_ih_{st}"], dtype=np.float32)
        b_hh = np.asarray(inputs[f"b_hh_{st}"], dtype=np.float32)
        W_eff = w_ih @ fc_w                          # [1024, 256]
        bias1 = w_ih @ fc_b + b_ih + b_hh            # [1024]
        sel = np.r_[0:D, 2 * D:4 * D]                # i, g, o rows
        We = np.ascontiguousarray(W_eff[sel].T)      # [256, 768]
        We[:, D:2 * D] *= 2.0                        # g-gate doubling
        Wh = np.ascontiguousarray((0.5 * (w_ih[:, :D] + w_hh)).T)  # [256, 1024]
        Wr = np.ascontiguousarray(w_ih[:, D:].T)     # [256, 1024]
        Wh[:, 2 * D:3 * D] *= 2.0
        Wr[:, 2 * D:3 * D] *= 2.0
        ws_e = _pow2_scale(We)
        ws_h = _pow2_scale(np.concatenate([Wh, Wr], axis=0))
        scales[f'ws_e_{st}'] = ws_e
        scales[f'ws_h_{st}'] = ws_h
        # per-slice activation biases (pre-multiplied by the tanh input
        # scale: 0.5 normally, 1.0 for the doubled g-gate)
        b1_sel = bias1[sel]                          # [768] i,g,o
        bias2 = b_ih + b_hh                          # [1024] i,f,g,o
        b1_cols = np.zeros((128, 6), np.float32)
        for m in range(6):
            f = 1.0 if m in (2, 3) else 0.5
            b1_cols[:, m] = f * b1_sel[m * 128:(m + 1) * 128]
        b2_cols = np.zeros((128, 8), np.float32)
        for m in range(8):
            f = 1.0 if m in (4, 5) else 0.5
            b2_cols[:, m] = f * bias2[m * 128:(m + 1) * 128]
        any_b |= bool(np.any(b1_cols != 0.0) or np.any(b2_cols != 0.0))
        per_stream[st] = (_f8(We * ws_e), _f8(Wh * ws_h), _f8(Wr * ws_h),
                          b1_cols, b2_cols)

    # out_w columns for the h-halves get the 0.5 compensation (h stored as 2h)
    ow = out_w.copy()
    ow[:, 0:D] *= 0.5
    ow[:, 2 * D:3 * D] *= 0.5
    ows = _pow2_scale(ow)
    scales['ows'] = ows
    outw8 = _f8(ow.T * ows)                          # [1024, 7]
    host_out_b = out_b

    # conversation -> (core, slot): sort by length desc, round-robin
    order = np.argsort(-lengths, kind="stable")
    assign = {}
    for rank, conv in enumerate(order):
        assign[int(conv)] = (rank % NCORE, rank // NCORE)
    order_lens = lengths[order]
    slot_lens = tuple(int(order_lens[8 * k]) for k in range(NCONV))

    # fp8-quantize the banks once (identical bytes for both layouts)
    import ml_dtypes
    xs8 = np.clip(x_s, -240.0, 240.0).astype(ml_dtypes.float8_e4m3fn).view(np.uint8)
    xp8 = np.clip(x_p, -240.0, 240.0).astype(ml_dtypes.float8_e4m3fn).view(np.uint8)
    one8 = int(np.array([1.0], dtype=ml_dtypes.float8_e4m3fn).view(np.uint8)[0])

    in_maps = []
    core_convs = []
    for core in range(NCORE):
        ids = [None] * NCONV
        for conv, (c, s) in assign.items():
            if c == core:
                ids[s] = conv
        core_convs.append(ids)
        m8 = np.zeros((NCONV, 128, 512), dtype=np.uint8)
        xns = xs8[:, ids, :].copy()      # [T_MAX, NCONV, D], u-major
        xnp = xp8[:, ids, :].copy()
        for s, conv in enumerate(ids):
            Lc = int(lengths[conv])
            valid = (np.arange(T_MAX) < Lc)
            m8[s, :, :] = np.where(valid, one8, 0).astype(np.uint8).reshape(
                4, 128).T.repeat(128, axis=1).reshape(128, 512)
            xns[Lc:, s, :] = 0
            xnp[Lc:, s, :] = 0
        im = {
            "xts": np.ascontiguousarray(
                xs8[:, ids, :].transpose(1, 2, 0).reshape(NCONV, 2, 128, T_MAX)),
            "xtp": np.ascontiguousarray(
                xp8[:, ids, :].transpose(1, 2, 0).reshape(NCONV, 2, 128, T_MAX)),
            "xns": np.ascontiguousarray(xns),
            "xnp": np.ascontiguousarray(xnp),
            "m8": m8,
            "outw": outw8,
        }
        for st in ("s", "p"):
            We8, Wh8, Wr8, b1c, b2c = per_stream[st]
            im[f"we_{st}"] = We8
            im[f"wh_{st}"] = Wh8
            im[f"wr_{st}"] = Wr8
            if any_b:
                im[f"b1_{st}"] = b1c
                im[f"b2_{st}"] = b2c
        in_maps.append(im)
    key = (any_b, slot_lens,
           tuple(sorted((k, float(v)) for k, v in scales.items())))
    return in_maps, core_convs, lengths, key, scales, host_out_b


def _gather(results, core_convs, lengths, out_b):
    """results: per-core {'out': [NCONV, T_MAX, C]} -> [sum(len), C]."""
    where = {}
    for core, ids in enumerate(core_convs):
        for slot, conv in enumerate(ids):
            where[conv] = (core, slot)
    chunks = []
    nz = bool(np.any(out_b != 0.0))
    for b in range(BATCH):
        core, slot = where[b]
        L = int(lengths[b])
        lg = results[core]["out"][slot, :L, :]
        if nz:
            # device log-softmax omitted out_b; log_softmax is shift-invariant
            # per row, so redo it with the bias added.
            lg = lg + out_b[None, :]
            lg = lg - np.log(np.exp(lg).sum(axis=1, keepdims=True))
        chunks.append(np.ascontiguousarray(lg))
    return np.concatenate(chunks, axis=0).astype(np.float32)


def _get_nc(key, scales):
    if key not in _BUILD_CACHE:
        _BUILD_CACHE[key] = _build(key[0], key[1], scales)
    return _BUILD_CACHE[key]


def kernel(**inputs):
    from concourse import bass_utils
    in_maps, core_convs, lengths, key, scales, out_b = _host_prep(inputs)
    nc = _get_nc(key, scales)
    res = bass_utils.run_bass_kernel_spmd(nc, in_maps, core_ids=list(range(NCORE)))
    return _gather(res.results, core_convs, lengths, out_b)


# revision 24
# speedup vs baseline: 1.9866x; 1.1380x over previous
"""DCRNCognition Trainium2 kernel — fp8 DoubleRow edition.

Self-contained: builds a Bass/Tile SPMD program for 8 NeuronCores, shards the
batch (conversation) axis across cores, runs via run_bass_kernel_spmd, and
gathers the valid positions on the host.

Math restructuring (identical to the verified baseline, rel err ~9e-7 in f32):
  - fc layer folded into step-1 LSTM gates; step-1 f-gate/c-init dead
  - step-2: gates2 = hs1 @ Wh.T + r1 @ Wr.T  (Wh = 0.5*(w_ih[:, :D]+w_hh))
  - softmax normalization deferred to r:  r = (X^T A) * (1/sum_u A)
  - sigmoid via tanh; h,c carried scaled by 2 (hs=2h, cs=2c)

Precision plan (validated on host: rel err ~6e-4 vs the 2e-2 gate):
  - ALL matmuls fp8 e4m3 with DoubleRow perf mode, fp32 PSUM accum.
  - weights pre-scaled by a power of 2 into fp8 range on the host; the
    compensation folds into the free activation `scale` operands.
  - g-gate weight rows doubled on host so every gate activation shares
    scale=0.5 -> one ACT instruction per gate pair.
  - masking without exp bias: invalid bank rows (u >= len) are zeroed on
    the host in the u-major layout (kills their r contribution) and the
    softmax denominator contracts A against a per-conversation 0/1 mask
    as the matmul stationary (kills them in the sum).
  - logits computed transposed ([t,C]): log-softmax along the free axis,
    one batched Ln at the end -> 2 ACT table loads total.

Scheduling: 4-stage software pipeline (F=gates1+cell1, B1=attention1,
B2=gates2+cell2, B3=attention2+logits) with a 3-conversation skew so every
engine queue always holds ready work. PSUM: two 4-bank pools (gates /
attention+logits).
"""
import os
import sys
sys.path.insert(0, '/opt/trn_rl_repo')

# run_bass_kernel_spmd executes through jax/PJRT on the axon-tunneled
# NeuronCores; a JAX_PLATFORMS=cpu pin would hide them.
if os.environ.get('JAX_PLATFORMS') == 'cpu' and 'jax' not in sys.modules:
    del os.environ['JAX_PLATFORMS']

import math
import numpy as np

T_MAX, BATCH, D, C = 512, 128, 256, 7
NCORE = 8
NCONV = BATCH // NCORE          # conversations per core

_BUILD_CACHE = {}


def _f8(x):
    """Host fp32 -> e4m3 bytes (clipped to the TRN-compatible +-240 range)."""
    import ml_dtypes
    return np.ascontiguousarray(
        np.clip(np.asarray(x, np.float32), -240.0, 240.0)
        .astype(ml_dtypes.float8_e4m3fn).view(np.uint8))


def _pow2_scale(w):
    s = float(np.std(w))
    if s == 0.0 or not np.isfinite(s):
        return 1.0
    return float(2.0 ** round(math.log2(4.0 / s)))


def _build(with_bias, slot_lens, scales):
    """Build + compile the SPMD Bass program. Returns the Bacc instance."""
    from contextlib import ExitStack
    import concourse.bacc as bacc
    import concourse.bass as bass  # noqa: F401
    from concourse import mybir, tile

    f32 = mybir.dt.float32
    bf16 = mybir.dt.bfloat16
    fp8 = mybir.dt.float8e4
    u8 = mybir.dt.uint8
    AF = mybir.ActivationFunctionType
    ALU = mybir.AluOpType
    AX = mybir.AxisListType
    PM = mybir.MatmulPerfMode.DoubleRow

    ws_e = {0: scales['ws_e_s'], 1: scales['ws_e_p']}
    ws_h = {0: scales['ws_h_s'], 1: scales['ws_h_p']}
    ows = scales['ows']

    nc = bacc.Bacc("TRN2", target_bir_lowering=False, debug=False,
                   num_devices=NCORE)

    def din(name, shape, dt):
        return nc.dram_tensor(name, shape, dt, kind="ExternalInput").ap()

    xt_d = {0: din("xts", [NCONV, 2, 128, T_MAX], u8),
            1: din("xtp", [NCONV, 2, 128, T_MAX], u8)}
    xn_d = {0: din("xns", [T_MAX, NCONV, D], u8),
            1: din("xnp", [T_MAX, NCONV, D], u8)}
    wdefs = {}
    for sti, st in enumerate(("s", "p")):
        wdefs[sti] = dict(
            we=din(f"we_{st}", [D, 768], u8),
            wh=din(f"wh_{st}", [D, 1024], u8),
            wr=din(f"wr_{st}", [D, 1024], u8),
            b1=din(f"b1_{st}", [128, 6], f32) if with_bias else None,
            b2=din(f"b2_{st}", [128, 8], f32) if with_bias else None,
        )
    m8_d = din("m8", [NCONV, 128, 512], u8)   # 0/1 row-validity, per ut block
    outw_d = din("outw", [4 * D, C], u8)
    out_d = nc.dram_tensor("out", [NCONV, T_MAX, C], f32,
                           kind="ExternalOutput").ap()

    with ExitStack() as ctx:
        tc = ctx.enter_context(tile.TileContext(nc))
        const = ctx.enter_context(tc.tile_pool(name="const", bufs=1))
        xpool = ctx.enter_context(tc.tile_pool(name="xpool", bufs=10))
        work = ctx.enter_context(tc.tile_pool(name="work", bufs=2))
        fpool = ctx.enter_context(tc.tile_pool(name="fpool", bufs=3))
        opool = ctx.enter_context(tc.tile_pool(name="opool", bufs=2))
        gp = ctx.enter_context(tc.tile_pool(name="gp", bufs=2, space="PSUM"))
        ep = ctx.enter_context(tc.tile_pool(name="ep", bufs=2, space="PSUM"))

        # ---- constants / weights --------------------------------------
        W = {}
        for sti, st in enumerate(("s", "p")):
            d = wdefs[sti]
            we_t = const.tile([128, 2, 768], fp8, name=f"we_t{st}")
            nc.sync.dma_start(out=we_t, in_=d["we"].bitcast(fp8).rearrange(
                "(kt p) m -> p kt m", p=128))
            wh_t = const.tile([128, 2, 1024], fp8, name=f"wh_t{st}")
            nc.sync.dma_start(out=wh_t, in_=d["wh"].bitcast(fp8).rearrange(
                "(kt p) m -> p kt m", p=128))
            wr_t = const.tile([128, 2, 1024], fp8, name=f"wr_t{st}")
            nc.sync.dma_start(out=wr_t, in_=d["wr"].bitcast(fp8).rearrange(
                "(kt p) m -> p kt m", p=128))
            b1_t = b2_t = None
            if with_bias:
                b1_t = const.tile([128, 6], f32, name=f"b1_t{st}")
                nc.sync.dma_start(out=b1_t, in_=d["b1"])
                b2_t = const.tile([128, 8], f32, name=f"b2_t{st}")
                nc.sync.dma_start(out=b2_t, in_=d["b2"])
            W[sti] = dict(we=we_t, wh=wh_t, wr=wr_t, b1=b1_t, b2=b2_t)
        m8_t = const.tile([128, NCONV, 4, 128], fp8, name="m8_t")
        nc.sync.dma_start(out=m8_t, in_=m8_d.bitcast(fp8).rearrange(
            "j p (b c) -> p j b c", b=4))
        outw_t = const.tile([128, 8, C], fp8, name="outw_t")
        nc.sync.dma_start(out=outw_t, in_=outw_d.bitcast(fp8).rearrange(
            "(kt p) c -> p kt c", p=128))
        lg_all = const.tile([128, NCONV, 4, C], f32, name="lg_all")
        s_all = const.tile([128, NCONV * 4], f32, name="s_all")
        lnS = const.tile([128, NCONV * 4], f32, name="lnS")

        def mm(ps, lhsT, rhs, start, stop, pm=PM):
            nc.tensor.matmul(ps, lhsT, rhs, start=start, stop=stop,
                             perf_mode=pm)

        def dims(j):
            Lv = int(slot_lens[j])
            # 16-aligned: DoubleRow LDWEIGHTS requires k-pair step % 16 == 0
            L = min(T_MAX, ((Lv + 15) // 16) * 16)
            UT = (Lv + 127) // 128
            return Lv, L, UT

        S = [None] * NCONV      # per-conv pipeline state

        def gate_act(pg_ap, out_ap, nsl, scale, bias_t, bcol):
            """[128,nsl,L] psum -> bf16; merged unless per-z biases needed."""
            if with_bias:
                for z in range(nsl):
                    nc.scalar.activation(out_ap[:, z, :], pg_ap[:, z, :],
                                         AF.Tanh, scale=scale,
                                         bias=bias_t[:, bcol + z:bcol + z + 1])
            else:
                nc.scalar.activation(out_ap, pg_ap, AF.Tanh, scale=scale)

        def emit_dma(j):
            Lv, L, UT = dims(j)
            LX = UT * 128
            xt_, xn_ = {}, {}
            for st in (0, 1):
                xt = xpool.tile([128, 2, LX], fp8, tag="xt", name=f"xt{j}_{st}")
                for kd in range(2):
                    nc.sync.dma_start(out=xt[:, kd, :],
                                      in_=xt_d[st].bitcast(fp8)[j, kd, :, 0:LX])
                xn = xpool.tile([128, 4, D], fp8, tag="xn", name=f"xn{j}_{st}")
                for ut in range(UT):
                    nc.sync.dma_start(
                        out=xn[:, ut, :],
                        in_=xn_d[st].bitcast(fp8)[ut * 128:(ut + 1) * 128, j, :])
                xt_[st], xn_[st] = xt, xn
            S[j] = dict(xt=xt_, xn=xn_)

        def emit_F(j):
            Lv, L, UT = dims(j)
            st_ = S[j]
            g1_ = {}
            for st in (0, 1):
                w = W[st]
                gts = []
                for gi in range(3):            # (i0,i1) (g0,g1) (o0,o1)
                    pg_t = gp.tile([128, 2, T_MAX], f32, tag="pg",
                                   name=f"pg1{j}_{st}_{gi}")
                    for z in range(2):
                        m = 2 * gi + z
                        mm(pg_t[:, z, 0:L], w["we"][:, :, m * 128:(m + 1) * 128],
                           st_['xt'][st][:, :, 0:L], True, True)
                    gt = work.tile([128, 2, L], bf16, tag="g1", bufs=10,
                                   name=f"g1{j}_{st}_{gi}")
                    gate_act(pg_t[:, :, 0:L], gt[:, :, :], 2, 0.5 / ws_e[st],
                             w["b1"], 2 * gi)
                    gts.append(gt)
                g1_[st] = gts
            cs1 = work.tile([128, 4, L], bf16, tag="cs", bufs=6, name=f"cs1{j}")
            for st in (0, 1):
                nc.vector.scalar_tensor_tensor(cs1[:, 2 * st:2 * st + 2, :],
                                               g1_[st][0][:, :, :], 1.0,
                                               g1_[st][1][:, :, :],
                                               ALU.add, ALU.mult)
            th1 = work.tile([128, 4, L], bf16, tag="th", bufs=3, name=f"th1{j}")
            nc.scalar.activation(th1[:, :, :], cs1[:, :, :], AF.Tanh, scale=0.5)
            hs1_ = {}
            for st in (0, 1):
                hs1 = work.tile([128, 2, L], fp8, tag="hs", bufs=12,
                                name=f"hs1{j}_{st}")
                nc.vector.scalar_tensor_tensor(hs1[:, :, :],
                                               g1_[st][2][:, :, :], 1.0,
                                               th1[:, 2 * st:2 * st + 2, :],
                                               ALU.add, ALU.mult)
                hs1_[st] = hs1
            st_.update(cs1=cs1, hs1=hs1_)

        def attention(j, st, hs_tile, step, L, UT):
            """A = exp(0.5*e); Z = 1/(m8 . A) — masking via m8/zeroed-xn."""
            xt = S[j]['xt'][st]
            A = work.tile([128, 4, L], fp8, tag="A", bufs=4,
                          name=f"A{j}_{st}_{step}")
            done = 0
            while done < UT:
                take = 2 if UT - done >= 2 else 1
                et = ep.tile([128, 2, T_MAX], f32, tag="e",
                             name=f"e{j}_{st}_{step}_{done}")
                for q in range(take):
                    ut = done + q
                    mm(et[:, q, 0:L], xt[:, :, ut * 128:(ut + 1) * 128],
                       hs_tile[:, :, :], True, True)
                nc.scalar.activation(A[:, done:done + take, :],
                                     et[:, 0:take, 0:L], AF.Exp, scale=0.5)
                done += take
            NPAIR, ODD = UT // 2, UT % 2
            pt = ep.tile([128, 2, T_MAX], f32, tag="e", name=f"ps{j}_{st}_{step}")
            for k in range(NPAIR):
                mm(pt[:, 0, 0:L], m8_t[:, j, 2 * k:2 * k + 2, :],
                   A[:, 2 * k:2 * k + 2, :], k == 0,
                   k == NPAIR - 1 and not ODD)
            if ODD:
                mm(pt[:, 0, 0:L], m8_t[:, j, UT - 1, :], A[:, UT - 1, :],
                   NPAIR == 0, True, pm=None)
            Z = work.tile([128, L], f32, tag="Z", bufs=4, name=f"Z{j}_{st}_{step}")
            nc.vector.reciprocal_approx_fast(Z[:, :], pt[:, 0, 0:L])
            return A, Z

        def r_apply(j, st, A, Z, out_tile, zoff, relu, step, L, UT):
            """out[:, zoff+dt, :] = (X^T A) * Z, optionally relu'd."""
            xn = S[j]['xn'][st]
            NPAIR, ODD = UT // 2, UT % 2
            for dt in range(2):
                rt = ep.tile([128, 2, T_MAX], f32, tag="e",
                             name=f"r{j}_{st}_{step}_{dt}")
                for k in range(NPAIR):
                    mm(rt[:, 0, 0:L],
                       xn[:, 2 * k:2 * k + 2, dt * 128:(dt + 1) * 128],
                       A[:, 2 * k:2 * k + 2, :], k == 0,
                       k == NPAIR - 1 and not ODD)
                if ODD:
                    mm(rt[:, 0, 0:L], xn[:, UT - 1, dt * 128:(dt + 1) * 128],
                       A[:, UT - 1, :], NPAIR == 0, True, pm=None)
                nc.vector.scalar_tensor_tensor(
                    out_tile[:, zoff + dt, :], rt[:, 0, 0:L],
                    0.0 if relu else 1.0, Z[:, :],
                    ALU.max if relu else ALU.mult, ALU.mult)

        def emit_B1(j):
            Lv, L, UT = dims(j)
            st_ = S[j]
            AZ1, r1_ = {}, {}
            for st in (0, 1):
                AZ1[st] = attention(j, st, st_['hs1'][st], 1, L, UT)
            for st in (0, 1):
                r1 = work.tile([128, 2, L], fp8, tag="r1", bufs=8,
                               name=f"r1{j}_{st}")
                r_apply(j, st, AZ1[st][0], AZ1[st][1], r1, 0, False, 1, L, UT)
                r1_[st] = r1
            st_.update(r1=r1_)

        def emit_B2(j):
            Lv, L, UT = dims(j)
            st_ = S[j]
            g2_ = {}
            for st in (0, 1):
                w = W[st]
                gts = []
                for gi in range(4):            # i, f, g, o pairs
                    pg_t = gp.tile([128, 2, T_MAX], f32, tag="pg",
                                   name=f"pg2{j}_{st}_{gi}")
                    for z in range(2):
                        m = 2 * gi + z
                        mm(pg_t[:, z, 0:L],
                           w["wh"][:, :, m * 128:(m + 1) * 128],
                           st_['hs1'][st][:, :, :], True, False)
                        mm(pg_t[:, z, 0:L],
                           w["wr"][:, :, m * 128:(m + 1) * 128],
                           st_['r1'][st][:, :, :], False, True)
                    gt = work.tile([128, 2, L], bf16, tag="g2", bufs=10,
                                   name=f"g2{j}_{st}_{gi}")
                    gate_act(pg_t[:, :, 0:L], gt[:, :, :], 2, 0.5 / ws_h[st],
                             w["b2"], 2 * gi)
                    gts.append(gt)
                g2_[st] = gts                  # [i, f, g, o]
            cs2 = work.tile([128, 4, L], bf16, tag="cs", bufs=6, name=f"cs2{j}")
            for st in (0, 1):
                gi2, gf2, gg2, go2 = g2_[st]
                t1 = work.tile([128, 2, L], bf16, tag="tmp", bufs=4,
                               name=f"t1{j}_{st}")
                nc.vector.scalar_tensor_tensor(t1[:, :, :], gf2[:, :, :], 1.0,
                                               st_['cs1'][:, 2 * st:2 * st + 2, :],
                                               ALU.add, ALU.mult)
                t2 = work.tile([128, 2, L], bf16, tag="tmp", bufs=4,
                               name=f"t2{j}_{st}")
                nc.vector.scalar_tensor_tensor(t2[:, :, :], gi2[:, :, :], 1.0,
                                               gg2[:, :, :], ALU.add, ALU.mult)
                nc.vector.scalar_tensor_tensor(cs2[:, 2 * st:2 * st + 2, :],
                                               t1[:, :, :], 0.5, t2[:, :, :],
                                               ALU.mult, ALU.add)
            th2 = work.tile([128, 4, L], bf16, tag="th", bufs=3, name=f"th2{j}")
            nc.scalar.activation(th2[:, :, :], cs2[:, :, :], AF.Tanh, scale=0.5)
            hs2_ = {}
            for st in (0, 1):
                hs2 = work.tile([128, 2, L], fp8, tag="hs", bufs=12,
                                name=f"hs2{j}_{st}")
                nc.vector.scalar_tensor_tensor(hs2[:, :, :],
                                               g2_[st][3][:, :, :], 1.0,
                                               th2[:, 2 * st:2 * st + 2, :],
                                               ALU.add, ALU.mult)
                hs2_[st] = hs2
            st_.update(hs2=hs2_)

        def emit_B3(j):
            Lv, L, UT = dims(j)
            st_ = S[j]
            ft_ = {}
            for st in (0, 1):
                A2, Z2 = attention(j, st, st_['hs2'][st], 2, L, UT)
                ft = fpool.tile([128, 4, L], fp8, tag=f"ft{st}", name=f"ft{j}_{st}")
                nc.vector.tensor_scalar_max(ft[:, 0:2, :],
                                            st_['hs2'][st][:, :, :], 0.0)
                r_apply(j, st, A2, Z2, ft, 2, True, 2, L, UT)
                ft_[st] = ft
            # logits transposed: [t, C] per 128-t chunk (8-wide slots in psum)
            lpt = ep.tile([128, 2, T_MAX], f32, tag="e", name=f"lp{j}")
            for cch in range(UT):
                ncch = min(128, L - cch * 128)
                for m, (ftile, zz) in enumerate(
                        ((ft_[0], 0), (ft_[0], 2), (ft_[1], 0), (ft_[1], 2))):
                    mm(lpt[0:ncch, 0, cch * 8:cch * 8 + C],
                       ftile[:, zz:zz + 2, cch * 128:cch * 128 + ncch],
                       outw_t[:, 2 * m:2 * m + 2, :], m == 0, m == 3)
            lgv = lpt[:, 0, 0:UT * 8].rearrange("p (u c) -> p u c", c=8)[:, :, 0:C]
            nc.vector.tensor_scalar_mul(lg_all[:, j, 0:UT, :], lgv, 1.0 / ows)
            elg = work.tile([128, 4, C], f32, tag="elg", name=f"elg{j}")
            nc.scalar.activation(elg[:, 0:UT, :], lgv, AF.Exp, scale=1.0 / ows)
            nc.vector.tensor_reduce(s_all[:, j * 4:j * 4 + UT],
                                    elg[:, 0:UT, :], AX.X, ALU.add)
            S[j] = None

        # ---- 4-stage software-pipelined main loop ----------------------
        emit_dma(0)
        for t in range(NCONV + 3):
            if t + 1 < NCONV:
                emit_dma(t + 1)
            if t >= 3:
                emit_B3(t - 3)
            if t >= 2 and t - 2 < NCONV:
                emit_B2(t - 2)
            if t >= 1 and t - 1 < NCONV:
                emit_B1(t - 1)
            if t < NCONV:
                emit_F(t)

        # ---- final: logp = lg - ln(rowsum) ----------------------------
        nc.scalar.activation(lnS[:, :], s_all[:, :], AF.Ln)
        for j in range(NCONV):
            UT = dims(j)[2]
            ot = opool.tile([128, 4, C], f32, tag="ot", name=f"ot{j}")
            for cch in range(UT):
                nc.gpsimd.tensor_scalar_sub(ot[:, cch, :], lg_all[:, j, cch, :],
                                            lnS[:, j * 4 + cch:j * 4 + cch + 1])
            nc.sync.dma_start(
                out=out_d[j].rearrange("(c p) k -> p c k", p=128)[:, 0:UT, :],
                in_=ot[:, 0:UT, :])

    nc.compile()
    return nc


def _host_prep(inputs):
    """Fold weights, quantize to fp8, pick conversation->core assignment."""
    x_s = np.asarray(inputs["input"], dtype=np.float32)
    x_p = np.asarray(inputs["speakers"], dtype=np.float32)
    lengths = np.asarray(inputs["utterance_lengths"]).astype(np.int64)
    fc_w = np.asarray(inputs["fc_w"], dtype=np.float32)
    fc_b = np.asarray(inputs["fc_b"], dtype=np.float32)
    out_w = np.asarray(inputs["out_w"], dtype=np.float32)
    out_b = np.asarray(inputs["out_b"], dtype=np.float32)

    per_stream = {}
    scales = {}
    any_b = False
    for st in ("s", "p"):
        w_ih = np.asarray(inputs[f"w_ih_{st}"], dtype=np.float32)
        w_hh = np.asarray(inputs[f"w_hh_{st}"], dtype=np.float32)
        b_ih = np.asarray(inputs[f"b_ih_{st}"], dtype=np.float32)
        b_hh = np.asarray(inputs[f"b_hh_{st}"], dtype=np.float32)
        W_eff = w_ih @ fc_w                          # [1024, 256]
        bias1 = w_ih @ fc_b + b_ih + b_hh            # [1024]
        sel = np.r_[0:D, 2 * D:4 * D]                # i, g, o rows
        We = np.ascontiguousarray(W_eff[sel].T)      # [256, 768]
        We[:, D:2 * D] *= 2.0                        # g-gate doubling
        Wh = np.ascontiguousarray((0.5 * (w_ih[:, :D] + w_hh)).T)  # [256, 1024]
        Wr = np.ascontiguousarray(w_ih[:, D:].T)     # [256, 1024]
        Wh[:, 2 * D:3 * D] *= 2.0
        Wr[:, 2 * D:3 * D] *= 2.0
        ws_e = _pow2_scale(We)
        ws_h = _pow2_scale(np.concatenate([Wh, Wr], axis=0))
        scales[f'ws_e_{st}'] = ws_e
        scales[f'ws_h_{st}'] = ws_h
        # per-slice activation biases (pre-multiplied by the tanh input
        # scale: 0.5 normally, 1.0 for the doubled g-gate)
        b1_sel = bias1[sel]                          # [768] i,g,o
        bias2 = b_ih + b_hh                          # [1024] i,f,g,o
        b1_cols = np.zeros((128, 6), np.float32)
        for m in range(6):
            f = 1.0 if m in (2, 3) else 0.5
            b1_cols[:, m] = f * b1_sel[m * 128:(m + 1) * 128]
        b2_cols = np.zeros((128, 8), np.float32)
        for m in range(8):
            f = 1.0 if m in (4, 5) else 0.5
            b2_cols[:, m] = f * bias2[m * 128:(m + 1) * 128]
        any_b |= bool(np.any(b1_cols != 0.0) or np.any(b2_cols != 0.0))
        per_stream[st] = (_f8(We * ws_e), _f8(Wh * ws_h), _f8(Wr * ws_h),
                          b1_cols, b2_cols)

    # out_w columns for the h-halves get the 0.5 compensation (h stored as 2h)
    ow = out_w.copy()
    ow[:, 0:D] *= 0.5
    ow[:, 2 * D:3 * D] *= 0.5
    ows = _pow2_scale(ow)
    scales['ows'] = ows
    outw8 = _f8(ow.T * ows)                          # [1024, 7]
    host_out_b = out_b

    # conversation -> (core, slot): sort by length desc, round-robin
    order = np.argsort(-lengths, kind="stable")
    assign = {}
    for rank, conv in enumerate(order):
        assign[int(conv)] = (rank % NCORE, rank // NCORE)
    order_lens = lengths[order]
    slot_lens = tuple(int(order_lens[8 * k]) for k in range(NCONV))

    # fp8-quantize the banks once (identical bytes for both layouts)
    import ml_dtypes
    xs8 = np.clip(x_s, -240.0, 240.0).astype(ml_dtypes.float8_e4m3fn).view(np.uint8)
    xp8 = np.clip(x_p, -240.0, 240.0).astype(ml_dtypes.float8_e4m3fn).view(np.uint8)
    one8 = int(np.array([1.0], dtype=ml_dtypes.float8_e4m3fn).view(np.uint8)[0])

    in_maps = []
    core_convs = []
    for core in range(NCORE):
        ids = [None] * NCONV
        for conv, (c, s) in assign.items():
            if c == core:
                ids[s] = conv
        core_convs.append(ids)
        m8 = np.zeros((NCONV, 128, 512), dtype=np.uint8)
        xns = xs8[:, ids, :].copy()      # [T_MAX, NCONV, D], u-major
        xnp = xp8[:, ids, :].copy()
        for s, conv in enumerate(ids):
            Lc = int(lengths[conv])
            valid = (np.arange(T_MAX) < Lc)
            m8[s, :, :] = np.where(valid, one8, 0).astype(np.uint8).reshape(
                4, 128).T.repeat(128, axis=1).reshape(128, 512)
            xns[Lc:, s, :] = 0
            xnp[Lc:, s, :] = 0
        im = {
            "xts": np.ascontiguousarray(
                xs8[:, ids, :].transpose(1, 2, 0).reshape(NCONV, 2, 128, T_MAX)),
            "xtp": np.ascontiguousarray(
                xp8[:, ids, :].transpose(1, 2, 0).reshape(NCONV, 2, 128, T_MAX)),
            "xns": np.ascontiguousarray(xns),
            "xnp": np.ascontiguousarray(xnp),
            "m8": m8,
            "outw": outw8,
        }
        for st in ("s", "p"):
            We8, Wh8, Wr8, b1c, b2c = per_stream[st]
            im[f"we_{st}"] = We8
            im[f"wh_{st}"] = Wh8
            im[f"wr_{st}"] = Wr8
            if any_b:
                im[f"b1_{st}"] = b1c
                im[f"b2_{st}"] = b2c
        in_maps.append(im)
    key = (any_b, slot_lens,
           tuple(sorted((k, float(v)) for k, v in scales.items())))
    return in_maps, core_convs, lengths, key, scales, host_out_b


def _gather(results, core_convs, lengths, out_b):
    """results: per-core {'out': [NCONV, T_MAX, C]} -> [sum(len), C]."""
    where = {}
    for core, ids in enumerate(core_convs):
        for slot, conv in enumerate(ids):
            where[conv] = (core, slot)
    chunks = []
    nz = bool(np.any(out_b != 0.0))
    for b in range(BATCH):
        core, slot = where[b]
        L = int(lengths[b])
        lg = results[core]["out"][slot, :L, :]
        if nz:
            # device log-softmax omitted out_b; log_softmax is shift-invariant
            # per row, so redo it with the bias added.
            lg = lg + out_b[None, :]
            lg = lg - np.log(np.exp(lg).sum(axis=1, keepdims=True))
        chunks.append(np.ascontiguousarray(lg))
    return np.concatenate(chunks, axis=0).astype(np.float32)


def _get_nc(key, scales):
    if key not in _BUILD_CACHE:
        _BUILD_CACHE[key] = _build(key[0], key[1], scales)
    return _BUILD_CACHE[key]


def kernel(**inputs):
    from concourse import bass_utils
    in_maps, core_convs, lengths, key, scales, out_b = _host_prep(inputs)
    nc = _get_nc(key, scales)
    res = bass_utils.run_bass_kernel_spmd(nc, in_maps, core_ids=list(range(NCORE)))
    return _gather(res.results, core_convs, lengths, out_b)
